# revision 74
# baseline (speedup 1.0000x reference)
"""GATv2 graph layer Bass kernel for TRN2 (SPMD across 8 NeuronCores).

v2 design (edge-parallel by dst range, windows of 120 dst nodes):
  - xsrc table (node_emb @ W_src, f16) built on device into DRAM (lo/hi
    halves as separate tensors so lo gathers overlap the hi build).
  - Per edge chunk (128 edges, one dst window): dma_gather xs rows; a
    combined one-hot ohx[e, 0:120]=dst, [120:128]=edge_type built by two
    broadcast is_equal ops; comb = ohxT^T @ [xdw_ext;emb] + I^T @ xs in
    PSUM (so xd gather/table is gone); ACT Prelu(0.2) -> lr f16.
  - logits = reduce32(lr * att_rep) on DVE; ex = exp(logits - 4) (shift
    cancels in softmax, keeps f16 finite); weighted = xs * ex.
  - Scatter: one matmul per chunk accumulates [sum_ex | sum(ex*xs)] into
    the window PSUM. Flush: attn-normalize, @W_out (gamma-scaled, beta
    via K=1 ones matmul), residual from transposed node cols, LayerNorm
    via bn_stats/bn_aggr + rstd = exp(-0.5*ln(var+eps)) (one ACT table).
"""
import ml_dtypes
import numpy as np
from contextlib import ExitStack
from dataclasses import dataclass

import concourse.bass as bass
import concourse.tile as tile
from concourse import bacc, mybir
from concourse.masks import make_identity

P = 128
HID = 128
H = 4
HD = 32
NET = 8
WIN = 120          # dst nodes per window (cols 120:128 select edge type)
EPS_LN = 1e-5
MAXCALL = 1024     # max idxs per dma_gather call
DEAD = -5.0        # padding marker (matches no iota column)
EXP_SHIFT = -4.0   # constant logit shift, cancels in softmax
BLK = 1024         # table build block cols


@dataclass
class Geo:
    N: int
    n_cores: int
    slab_w: int = 4

    @property
    def npc(self):
        return self.N // self.n_cores

    @property
    def nw(self):
        return (self.npc + WIN - 1) // WIN

    @property
    def nslab(self):
        return (self.nw + self.slab_w - 1) // self.slab_w

    # table geometry (n_lo/n_hi) lives in sched: the xsrc table holds only
    # the src rows actually referenced by each core's edges (compacted)


def wrap_idx(idx, cols):
    n = idx.shape[0]
    assert n % 16 == 0
    w = np.zeros((P, cols), dtype=np.int16)
    if n:
        t16 = idx.reshape(n // 16, 16).T
        for g in range(8):
            w[g * 16:(g + 1) * 16, :n // 16] = t16
    return w


def host_prep(g: Geo, node_embeddings, edge_index, edge_type, task_embedding,
              W_src, b_src, W_dst, b_dst, edge_emb, att,
              W_out, b_out, norm_w, norm_b, W_film, b_film):
    """Pure index work + tiny constant folding; all O(N*HID)/O(E*HID)
    float math runs on device."""
    src = np.asarray(edge_index[0], dtype=np.int64)
    dst = np.asarray(edge_index[1], dtype=np.int64)
    et = np.asarray(edge_type, dtype=np.int64)
    npc, nw = g.npc, g.nw

    order = np.argsort(dst, kind="stable")
    src, dst, et = src[order], dst[order], et[order]
    core_of = dst // npc

    # compact per-core src tables: only rows this core's edges reference
    used = [np.unique(src[core_of == c]) for c in range(g.n_cores)]
    n_used_max = max(len(u) for u in used)
    n_lo = ((n_used_max + 1) // 2 + BLK - 1) // BLK * BLK
    n_hi = max(BLK, ((n_used_max - min(n_used_max, n_lo)) + BLK - 1)
               // BLK * BLK)
    assert n_lo <= 32767 and n_hi <= 32767

    buckets = {}
    for c in range(g.n_cores):
        m = core_of == c
        pos = np.searchsorted(used[c], src[m])
        cd, ce = dst[m] - c * npc, et[m]
        for w in range(nw):
            wm = (cd // WIN) == w
            ws_, wd, we = pos[wm], cd[wm] - w * WIN, ce[wm]
            lo = ws_ < n_lo
            buckets[(c, w, 0)] = (ws_[lo], wd[lo], we[lo])
            buckets[(c, w, 1)] = (ws_[~lo] - n_lo, wd[~lo], we[~lo])

    caps = np.zeros((nw, 2), dtype=np.int64)
    for w in range(nw):
        for h in range(2):
            mx = max(len(buckets[(c, w, h)][0]) for c in range(g.n_cores))
            caps[w, h] = (mx + P - 1) // P
        if caps[w, 0] + caps[w, 1] == 0:
            caps[w, 0] = 1          # ensure every window PSUM gets reset

    sched_slabs = []
    total_chunks = 0
    for s in range(g.nslab):
        ws = list(range(s * g.slab_w, min((s + 1) * g.slab_w, nw)))
        chunks = []            # (wl, half, slot)
        calls = {0: [], 1: []}
        slot = 0
        for h in (0, 1):
            run = 0
            run_start = slot
            for w in ws:
                for _ in range(caps[w, h]):
                    chunks.append((w - ws[0], h, slot))
                    slot += 1
                    run += P
                    if run == MAXCALL:
                        calls[h].append((run_start, run))
                        run, run_start = 0, slot
            if run:
                calls[h].append((run_start, run))
        sched_slabs.append(dict(windows=ws, chunks=chunks, calls=calls,
                                chunk0=total_chunks))
        total_chunks += len(chunks)

    lo_cols = max(16, sum(n for sl in sched_slabs
                          for (_, n) in sl["calls"][0]) // 16)
    hi_cols = max(16, sum(n for sl in sched_slabs
                          for (_, n) in sl["calls"][1]) // 16)

    # ---- shared constants -------------------------------------------------
    node_f16 = np.asarray(node_embeddings, np.float32).astype(np.float16)
    emb_eff = (np.asarray(edge_emb, np.float64)
               + np.asarray(b_src, np.float64)[None, :]
               + np.asarray(b_dst, np.float64)[None, :]).astype(np.float16)

    consts = dict(
        W_src=np.asarray(W_src, np.float32).astype(np.float16),
        W_dst=np.asarray(W_dst, np.float32).astype(np.float16),
        W_out=np.asarray(W_out, np.float32).astype(np.float16),
        W_film=np.asarray(W_film, np.float32).astype(np.float16),
        b_film=np.asarray(b_film, np.float32).reshape(1, 2 * HID),
        b_out=np.asarray(b_out, np.float32).reshape(1, HID),
        task=np.asarray(task_embedding, np.float32).reshape(HID, 1)
            .astype(np.float16),
        emb8=emb_eff,                                        # [8, HID] f16
        att_row=np.asarray(att, np.float32).reshape(1, HID).astype(np.float16),
    )
    skip_norm = bool(np.all(np.asarray(norm_w) == 1.0)
                     and np.all(np.asarray(norm_b) == 0.0))
    if not skip_norm:
        consts["normw"] = np.asarray(norm_w, np.float32).reshape(1, HID)
        consts["normb"] = np.asarray(norm_b, np.float32).reshape(1, HID)

    # ---- per-core arrays --------------------------------------------------
    node_f32 = np.asarray(node_embeddings, np.float32)
    in_maps = []
    for c in range(g.n_cores):
        lo_l, hi_l = [], []
        dstr = np.full((P, total_chunks), DEAD, dtype=np.float16)
        etc = np.full((P, total_chunks), DEAD, dtype=np.float16)
        ci = 0
        for sl in sched_slabs:
            ws0 = sl["windows"][0]
            nth = {}
            for (wl, h, slot) in sl["chunks"]:
                w = ws0 + wl
                es, ed, ee = buckets[(c, w, h)]
                k = nth.get((wl, h), 0)
                nth[(wl, h)] = k + 1
                sl_src = np.zeros(P, dtype=np.int64)
                n = min(P, max(0, len(es) - k * P))
                if n > 0:
                    sl_src[:n] = es[k * P:k * P + n]
                    dstr[:n, ci] = ed[k * P:k * P + n]
                    etc[:n, ci] = WIN + ee[k * P:k * P + n]
                (lo_l if h == 0 else hi_l).append(sl_src)
                ci += 1
        lo_i = (np.concatenate(lo_l) if lo_l else np.zeros(0, np.int64))
        hi_i = (np.concatenate(hi_l) if hi_l else np.zeros(0, np.int64))
        assert lo_i.max(initial=0) < n_lo
        assert hi_i.max(initial=0) < n_hi

        # host-built one-hots. ohT [code, slot*128+e] feeds the comb matmul
        # lhsT directly; oh [e, slot*128+code] feeds the scatter lhsT.
        ohT = np.zeros((P, total_chunks * P), dtype=ml_dtypes.float8_e4m3fn)
        oh = np.zeros((P, total_chunks * P), dtype=ml_dtypes.float8_e4m3fn)
        pg, cg = np.meshgrid(np.arange(P), np.arange(total_chunks),
                             indexing="ij")
        for arr in (dstr, etc):
            v = arr >= 0
            code = arr[v].astype(np.int64)
            ohT[code, cg[v] * P + pg[v]] = 1.0
            oh[pg[v], cg[v] * P + code] = 1.0

        own = np.zeros((nw * WIN, HID), dtype=np.float16)
        own[:npc] = node_f32[c * npc:(c + 1) * npc].astype(np.float16)

        m = dict(consts)
        # per-core compacted src-node table (transposed), zero-padded
        nodeT = np.zeros((HID, n_lo + n_hi), dtype=np.float16)
        nodeT[:, :len(used[c])] = node_f16[used[c]].T
        m["nodeT"] = nodeT
        # transposed own nodes, 128 cols per window (last 8 zero: the emb
        # rows of xdw_ext ride in via a second accumulating matmul)
        ownT = np.zeros((HID, nw * P), dtype=np.float16)
        ot = own.T  # [HID, nw*WIN]
        for w in range(nw):
            ownT[:, w * P:w * P + WIN] = ot[:, w * WIN:(w + 1) * WIN]
        m["node_own_T"] = ownT
        m["node_own_w"] = np.ascontiguousarray(
            own.reshape(nw, WIN, HID).transpose(1, 0, 2))    # [WIN, nw, HID]
        m["lo_idx"] = wrap_idx(lo_i.astype(np.int16), lo_cols)
        m["hi_idx"] = wrap_idx(hi_i.astype(np.int16), hi_cols)
        m["ohT_in"] = ohT
        m["oh_in"] = oh
        in_maps.append(m)

    sched = dict(slabs=sched_slabs, caps=caps, total_chunks=total_chunks,
                 n_lo=n_lo, n_hi=n_hi,
                 lo_cols=lo_cols, hi_cols=hi_cols, skip_norm=skip_norm)
    return sched, in_maps


def build_program(g: Geo, sched, debug=False):
    nc = bacc.Bacc("TRN2", target_bir_lowering=False, debug=False,
                   num_devices=g.n_cores, num_swdge_queues=4)
    f16, f32 = mybir.dt.float16, mybir.dt.float32
    AF = mybir.ActivationFunctionType
    OP = mybir.AluOpType
    npc, nw = g.npc, g.nw
    total_chunks = sched["total_chunks"]
    lo_cols, hi_cols = sched["lo_cols"], sched["hi_cols"]
    n_lo = sched["n_lo"]
    n_hi = sched["n_hi"]

    def din(name, shape, dt):
        return nc.dram_tensor(name, shape, dt, kind="ExternalInput").ap()

    nodeT = din("nodeT", [HID, n_lo + n_hi], f16)
    node_own_T = din("node_own_T", [HID, nw * P], f16)
    node_own_w = din("node_own_w", [WIN, nw, HID], f16)
    W_src = din("W_src", [HID, HID], f16)
    W_dst = din("W_dst", [HID, HID], f16)
    W_out = din("W_out", [HID, HID], f16)
    W_film = din("W_film", [HID, 2 * HID], f16)
    b_film = din("b_film", [1, 2 * HID], f32)
    b_out = din("b_out", [1, HID], f32)
    task = din("task", [HID, 1], f16)
    emb8 = din("emb8", [NET, HID], f16)
    att_row = din("att_row", [1, HID], f16)
    lo_idx = din("lo_idx", [P, lo_cols], mybir.dt.int16)
    hi_idx = din("hi_idx", [P, hi_cols], mybir.dt.int16)
    f8 = mybir.dt.float8e4
    ohT_in = din("ohT_in", [P, total_chunks * P], f8)
    oh_in = din("oh_in", [P, total_chunks * P], f8)
    out = nc.dram_tensor("out", [npc, HID], f32, kind="ExternalOutput").ap()

    xsrc_lo = nc.dram_tensor("xsrc_lo", [n_lo, HID], f16).ap()
    xsrc_hi = nc.dram_tensor("xsrc_hi", [n_hi, HID], f16).ap()

    if debug:
        cmax = max(len(sl["chunks"]) for sl in sched["slabs"])
        nsl = len(sched["slabs"])
        dbg_xs = nc.dram_tensor("dbg_xs", [nsl, P, cmax, HID], f16,
                                kind="ExternalOutput").ap()
        dbg_ohx = nc.dram_tensor("dbg_ohx", [nsl, P, P, cmax], f16,
                                 kind="ExternalOutput").ap()
        dbg_lr = nc.dram_tensor("dbg_lr", [nsl, P, cmax, HID], f16,
                                kind="ExternalOutput").ap()
        dbg_rhs = nc.dram_tensor("dbg_rhs", [nsl, P, cmax, 4 + HID], f16,
                                 kind="ExternalOutput").ap()
        dbg_xdw = nc.dram_tensor("dbg_xdw", [nw, P, HID], f16,
                                 kind="ExternalOutput").ap()
        dbg_win = nc.dram_tensor("dbg_win", [nw, WIN, 4 + HID], f32,
                                 kind="ExternalOutput").ap()
        dbg_y = nc.dram_tensor("dbg_y", [nw, WIN, HID], f32,
                               kind="ExternalOutput").ap()

    with tile.TileContext(nc, trace_sim=False) as tc, ExitStack() as ctx:
        cpool = ctx.enter_context(tc.tile_pool(name="consts", bufs=1))
        bpool = ctx.enter_context(tc.tile_pool(name="build", bufs=3))
        xpool = ctx.enter_context(tc.tile_pool(name="xdw", bufs=2 * g.slab_w))
        psW = ctx.enter_context(tc.tile_pool(name="psW", bufs=1, space="PSUM"))
        spool = ctx.enter_context(tc.tile_pool(name="slab", bufs=2))
        gpool = ctx.enter_context(tc.tile_pool(name="grp", bufs=3))
        psG = ctx.enter_context(tc.tile_pool(name="psG", bufs=1, space="PSUM"))
        psC = ctx.enter_context(tc.tile_pool(name="psC", bufs=2, space="PSUM"))
        psA = ctx.enter_context(tc.tile_pool(name="psA", bufs=2, space="PSUM"))
        fpool = ctx.enter_context(tc.tile_pool(name="flush", bufs=2))
        ypool = ctx.enter_context(tc.tile_pool(name="yout", bufs=1))

        # ---- constants ----------------------------------------------------
        ident = cpool.tile([P, P], f16)
        make_identity(nc, ident[:])
        iota16 = cpool.tile([P, P], mybir.dt.int16)
        nc.gpsimd.iota(iota16[:], pattern=[[1, P]], base=0, channel_multiplier=0)
        iota = cpool.tile([P, P], f16)
        nc.vector.tensor_copy(iota[:], iota16[:])
        ones_row = cpool.tile([1, P], f16)
        nc.vector.memset(ones_row[:], 1.0)
        eps_col = cpool.tile([P, 1], f32)
        nc.vector.memset(eps_col[:], EPS_LN)
        shift_col = cpool.tile([P, 1], f32)
        nc.vector.memset(shift_col[:], EXP_SHIFT)

        Ws = cpool.tile([HID, HID], f16)
        nc.sync.dma_start(Ws[:], W_src[:])
        Wd = cpool.tile([HID, HID], f16)
        nc.sync.dma_start(Wd[:], W_dst[:])
        Wo = cpool.tile([HID, HID], f16)
        nc.sync.dma_start(Wo[:], W_out[:])
        Wf = cpool.tile([HID, 2 * HID], f16)
        nc.sync.dma_start(Wf[:], W_film[:])
        emb_sb = cpool.tile([NET, HID], f16)
        nc.sync.dma_start(emb_sb[:], emb8[:])
        att_sb = cpool.tile([1, HID], f16)
        nc.sync.dma_start(att_sb[:], att_row[:])
        task_sb = cpool.tile([HID, 1], f16)
        nc.sync.dma_start(task_sb[:], task[:])
        bfilm_sb = cpool.tile([1, 2 * HID], f32)
        nc.sync.dma_start(bfilm_sb[:], b_film[:])
        bout_sb = cpool.tile([1, HID], f32)
        nc.sync.dma_start(bout_sb[:], b_out[:])

        # residual rows, resident: node_own_sb[p, w, :] = own node w*WIN+p
        node_own_sb = cpool.tile([WIN, nw, HID], f16, tag="nodeown")
        nc.sync.dma_start(node_own_sb[:], node_own_w[:])
        # transposed own nodes, resident (feeds the per-window x_dst matmul)
        noT_all = cpool.tile([HID, nw * P], f16, tag="noT")
        nc.sync.dma_start(noT_all[:], node_own_T[:])
        # sel8[k, j] = (j == WIN + k): routes emb rows into xdw PSUM rows
        # WIN..127 via an accumulating K=8 matmul
        iotaP = cpool.tile([P, 1], mybir.dt.int16)
        nc.gpsimd.iota(iotaP[:], pattern=[[0, 1]], base=0, channel_multiplier=1)
        col8 = cpool.tile([P, 1], f32)
        nc.vector.tensor_scalar(col8[:], iotaP[:], float(WIN), None, OP.add)
        sel8 = cpool.tile([NET, P], f16)
        nc.vector.tensor_scalar(sel8[:], iota[0:NET, :], col8[0:NET, :], None,
                                OP.is_equal)

        lo_sb = cpool.tile([P, lo_cols], mybir.dt.int16, tag="loidx")
        nc.sync.dma_start(lo_sb[:], lo_idx[:])
        hi_sb = cpool.tile([P, hi_cols], mybir.dt.int16, tag="hiidx")
        nc.sync.dma_start(hi_sb[:], hi_idx[:])

        # ---- FiLM ---------------------------------------------------------
        ps_f = psW.tile([1, 2 * HID], f32, space="PSUM", tag="pw")
        nc.tensor.matmul(out=ps_f[:], lhsT=task_sb[:], rhs=Wf[:],
                         start=True, stop=True)
        film = cpool.tile([1, 2 * HID], f32)
        nc.vector.tensor_add(film[:], ps_f[:], bfilm_sb[:])
        gam_t = cpool.tile([1, HID], f32)
        nc.scalar.activation(gam_t[:], film[:, :HID], AF.Tanh)
        gam16 = cpool.tile([1, HID], f16)
        nc.vector.tensor_scalar(gam16[:], gam_t[:], 0.5, 1.0, OP.mult, OP.add)
        tmpb = cpool.tile([1, HID], f32)
        nc.vector.tensor_mul(tmpb[:], bout_sb[:], gam16[:])
        beta16 = cpool.tile([1, HID], f16)
        nc.vector.tensor_add(beta16[:], tmpb[:], film[:, HID:])
        ps_g = psW.tile([P, HID], f32, space="PSUM", tag="pw")
        nc.tensor.matmul(out=ps_g[:], lhsT=ones_row[:], rhs=gam16[:],
                         start=True, stop=True)
        gam_rep = cpool.tile([P, HID], f16)
        nc.vector.tensor_copy(gam_rep[:], ps_g[:])
        Wosc = cpool.tile([HID, HID], f16)
        nc.vector.tensor_mul(Wosc[:], Wo[:], gam_rep[:])
        ps_a = psW.tile([P, HID], f32, space="PSUM", tag="pw")
        nc.tensor.matmul(out=ps_a[:], lhsT=ones_row[:], rhs=att_sb[:],
                         start=True, stop=True)
        att_rep = cpool.tile([P, HID], f16)
        nc.vector.tensor_copy(att_rep[:], ps_a[:])

        if not sched["skip_norm"]:
            nw_dr = din("normw", [1, HID], f32)
            nb_dr = din("normb", [1, HID], f32)
            nw_sb = cpool.tile([1, HID], f32)
            nc.sync.dma_start(nw_sb[:], nw_dr[:])
            nb_sb = cpool.tile([1, HID], f32)
            nc.sync.dma_start(nb_sb[:], nb_dr[:])
            ones32 = cpool.tile([1, P], f32)
            nc.vector.memset(ones32[:], 1.0)
            ps_w = psW.tile([P, HID], f32, space="PSUM", tag="pw")
            nc.tensor.matmul(out=ps_w[:], lhsT=ones32[:], rhs=nw_sb[:],
                             start=True, stop=True)
            w_rep = cpool.tile([P, HID], f32)
            nc.vector.tensor_copy(w_rep[:], ps_w[:])
            ps_b = psW.tile([P, HID], f32, space="PSUM", tag="pw")
            nc.tensor.matmul(out=ps_b[:], lhsT=ones32[:], rhs=nb_sb[:],
                             start=True, stop=True)
            b_rep = cpool.tile([P, HID], f32)
            nc.vector.tensor_copy(b_rep[:], ps_b[:])

        # ---- xsrc table (lo half first so lo gathers can start) -----------
        for b in range((n_lo + n_hi) // BLK):
            nt = bpool.tile([HID, BLK], f16, tag="nt")
            nc.sync.dma_start(nt[:], nodeT[:, b * BLK:(b + 1) * BLK])
            xt = bpool.tile([P, 8, HID], f16, tag="xt")
            for sub in range(2):
                # rides the 2-bank comb pool (same shape/tag) so the block
                # pipeline isn't strangled by a single PSUM bank
                ps = psC.tile([P, 4, HID], f32, space="PSUM", tag="pc")
                for j in range(4):
                    nc.tensor.matmul(
                        out=ps[:, j, :],
                        lhsT=nt[:, sub * 512 + j * P: sub * 512 + (j + 1) * P],
                        rhs=Ws[:], start=True, stop=True,
                        skip_group_check=True)
                if sub == 0:
                    nc.scalar.activation(xt[:, :4, :], ps[:], AF.Copy)
                else:
                    nc.vector.tensor_copy(xt[:, 4:, :], ps[:])
            r0 = b * BLK
            tab, base = (xsrc_lo, 0) if r0 < n_lo else (xsrc_hi, n_lo)
            # table writes ride the ACT hwdge queue; SP carries the reads
            nc.scalar.dma_start(
                tab[r0 - base:r0 - base + BLK, :]
                .rearrange("(s p) h -> p s h", p=P), xt[:])

        # ---- slabs --------------------------------------------------------
        yall = ypool.tile([WIN, nw, HID], f32, tag="yall")
        mvall = ypool.tile([WIN, nw, 2], f32, tag="mvall")
        off16 = {0: 0, 1: 0}
        qn = [0]

        def nextq():
            qn[0] = (qn[0] + 1) % 4
            return qn[0]

        UNIT = 16
        slab_state = {}

        def stage_prep(s):
            """xdw_ext tiles (rows 0:WIN = x_dst via W_dst, WIN:128 = emb)."""
            sl = sched["slabs"][s]
            xdw_t = []
            for wl, w in enumerate(sl["windows"]):
                px = psW.tile([P, HID], f32, space="PSUM", tag="pw")
                nc.tensor.matmul(out=px[:],
                                 lhsT=noT_all[:, w * P:(w + 1) * P],
                                 rhs=Wd[:], start=True, stop=False,
                                 skip_group_check=True)
                nc.tensor.matmul(out=px[:], lhsT=sel8[:], rhs=emb_sb[:],
                                 start=False, stop=True,
                                 skip_group_check=True)
                xdw = xpool.tile([P, HID], f16, tag="xdw",
                                 name=f"xdw{s}_{wl}")
                nc.scalar.activation(xdw[:], px[:], AF.Copy)
                xdw_t.append(xdw)
            C = len(sl["chunks"])
            win_ps = psA.tile([WIN, g.slab_w, 256], f32, space="PSUM",
                              tag="win", name=f"win{s}")
            nc.scalar.memzero(win_ps[:])
            c0 = sl["chunk0"]
            ohxT = spool.tile([P, C, P], mybir.dt.float8e4, tag="ohT",
                              name=f"ohT{s}")
            nc.sync.dma_start(ohxT[:],
                              ohT_in[:, c0 * P:(c0 + C) * P]
                              .rearrange("p (c e) -> p c e", e=P))
            slab_state[s] = dict(
                xdw_t=xdw_t,
                xs_t=spool.tile([P, C, HID], f16, tag="xs", name=f"xs{s}"),
                ohx=spool.tile([P, C, P], mybir.dt.float8e4, tag="ohx",
                               name=f"ohx{s}"),
                win_ps=win_ps, ohxT=ohxT,
            )

        def stage_gather(s):
            sl = sched["slabs"][s]
            xs_t = slab_state[s]["xs_t"]
            for h, (idx_sb, tab, tabn) in enumerate(
                    ((lo_sb, xsrc_lo, n_lo), (hi_sb, xsrc_hi, n_hi))):
                for (slot_off, n) in sl["calls"][h]:
                    if n == 0:
                        continue
                    nc.gpsimd.dma_gather(
                        out_ap=xs_t[:, slot_off:slot_off + n // P, :],
                        in_ap=tab[0:tabn, :],
                        idxs_ap=idx_sb[:, off16[h]:off16[h] + n // 16],
                        num_idxs=n, num_idxs_reg=n, elem_size=HID,
                        single_packet=(n <= 1024), queue_num=nextq(),
                    )
                    off16[h] += n // 16

        def stage_ohx(s):
            """Edge-major one-hot (scatter lhsT), host-built, via DMA."""
            sl = sched["slabs"][s]
            C = len(sl["chunks"])
            c0 = sl["chunk0"]
            ohx = slab_state[s]["ohx"]
            nc.sync.dma_start(ohx[:],
                              oh_in[:, c0 * P:(c0 + C) * P]
                              .rearrange("p (c e) -> p c e", e=P))

        def stage_units(s):
            sl = sched["slabs"][s]
            chunks = sl["chunks"]
            C = len(chunks)
            st = slab_state[s]
            xs_t, ohx, xdw_t = st["xs_t"], st["ohx"], st["xdw_t"]
            lr = spool.tile([P, C, HID], f16, tag="lr", name=f"lr{s}")
            rhs_t = spool.tile([P, C, 4 + HID], f16, tag="rhs", name=f"rhs{s}")
            logits = gpool.tile([P, C, H], f16, tag="logit", name=f"lg{s}")
            st["rhs_t"] = rhs_t
            st["lr"] = lr

            ohxT = st["ohxT"]

            def unit_pe(u0, un):
                # comb psum + Prelu, per 4-chunk psum group (ohxT host-built)
                for g0 in range(u0, u0 + un, 4):
                    gn = min(4, u0 + un - g0)
                    psc = psC.tile([P, 4, HID], f32, space="PSUM", tag="pc")
                    for k in range(gn):
                        wl = chunks[g0 + k][0]
                        nc.tensor.matmul(out=psc[:, k, :],
                                         lhsT=ohxT[:, g0 + k, :],
                                         rhs=xdw_t[wl][:], start=True,
                                         stop=False, skip_group_check=True)
                        nc.tensor.matmul(out=psc[:, k, :], lhsT=ident[:],
                                         rhs=xs_t[:, g0 + k, :], start=False,
                                         stop=True, skip_group_check=True)
                    nc.scalar.activation(lr[:, g0:g0 + gn, :], psc[:, :gn, :],
                                         AF.Prelu, alpha=0.2)

            def unit_post(u0, un):
                # logits / softmax-numerator / weighted messages
                lr4 = lr[:, u0:u0 + un, :].rearrange(
                    "p c (h d) -> p c h d", h=H)
                wgt4 = rhs_t[:, u0:u0 + un, 4:4 + HID].rearrange(
                    "p c (h d) -> p c h d", h=H)
                nc.vector.tensor_mul(
                    wgt4, lr4,
                    att_rep[:].rearrange("p (h d) -> p h d", h=H).unsqueeze(1)
                    .broadcast_to([P, un, H, HD]))
                # tree-fold before the (slow-mode) reduce: adds run at 2x
                nc.vector.tensor_add(wgt4[:, :, :, 0:16], wgt4[:, :, :, 0:16],
                                     wgt4[:, :, :, 16:32])
                nc.vector.tensor_add(wgt4[:, :, :, 0:8], wgt4[:, :, :, 0:8],
                                     wgt4[:, :, :, 8:16])
                with nc.allow_low_precision(reason="logits |x|<12, f16 ok"):
                    nc.vector.tensor_reduce(
                        logits[:, u0:u0 + un, :], wgt4[:, :, :, 0:8],
                        axis=mybir.AxisListType.X, op=OP.add)
                # exp with broadcast INPUT: ex_rep comes out pre-expanded to
                # 32 cols/head so the weighted mul keeps packed operands
                ex_rep = gpool.tile([P, UNIT, HID], f16, tag="exrep")
                nc.scalar.activation(
                    ex_rep[:, :un, :],
                    logits[:, u0:u0 + un, :].unsqueeze(3)
                    .broadcast_to([P, un, H, HD]),
                    AF.Exp, bias=shift_col[:])
                nc.vector.tensor_copy(
                    rhs_t[:, u0:u0 + un, 0:4],
                    ex_rep[:, :un, :].rearrange(
                        "p c (h d) -> p c h d", h=H)[:, :, :, 0])
                nc.vector.tensor_mul(
                    rhs_t[:, u0:u0 + un, 4:4 + HID],
                    xs_t[:, u0:u0 + un, :], ex_rep[:, :un, :])

            # scatter: the bank was memset in stage_prep; all matmuls
            # accumulate (start=False), stop=True on each window's last
            # chunk. Interleaving accumulate-mode writes in one bank is
            # safe (unlike interleaved start=True groups, which corrupt).
            win_ps = st["win_ps"]
            last_slot = {}
            for (wl, h, slot) in chunks:
                last_slot[wl] = slot

            def unit_scatter(u0, un):
                for slot in range(u0, u0 + un):
                    wl = chunks[slot][0]
                    nc.tensor.matmul(out=win_ps[:, wl, 0:4 + HID],
                                     lhsT=ohx[:, slot, 0:WIN],
                                     rhs=rhs_t[:, slot, :],
                                     start=False,
                                     stop=(last_slot[wl] == slot),
                                     skip_group_check=True)

            units = [(u0, min(UNIT, C - u0)) for u0 in range(0, C, UNIT)]
            pend = []
            for (u0, un) in units:
                unit_pe(u0, un)
                if pend:
                    unit_post(*pend[-1])
                if len(pend) >= 2:
                    unit_scatter(*pend[-2])  # two-unit lag: zero PE stall
                pend.append((u0, un))
            unit_post(*pend[-1])
            for (u0, un) in pend[-2:]:
                unit_scatter(u0, un)

        def stage_flush(s):
            sl = sched["slabs"][s]
            ws = sl["windows"]
            nwin = len(ws)
            st = slab_state[s]
            win_ps = st["win_ps"]
            w0 = ws[0]
            if debug:
                C = len(sl["chunks"])
                nc.sync.dma_start(dbg_xs[s, :, :C, :], st["xs_t"][:])
                nc.sync.dma_start(dbg_ohx[s, :, :, :C],
                                  st["ohx"][:].rearrange("p c e -> p e c"))
                nc.sync.dma_start(dbg_lr[s, :, :C, :], st["lr"][:])
                nc.sync.dma_start(dbg_rhs[s, :, :C, :], st["rhs_t"][:])
                for wl, w in enumerate(ws):
                    nc.sync.dma_start(dbg_xdw[w], st["xdw_t"][wl][:])
                    dbw = fpool.tile([WIN, 4 + HID], f32, tag="dbw")
                    nc.vector.tensor_copy(dbw[:], win_ps[:, wl, :])
                    nc.sync.dma_start(dbg_win[w], dbw[:])
            # attention normalization, batched across the slab's windows
            sums = fpool.tile([WIN, g.slab_w, 4], f32, tag="sums")
            nc.vector.tensor_scalar(sums[:, :nwin, :], win_ps[:, :nwin, 0:4],
                                    1e-12, None, OP.max)
            rec = fpool.tile([WIN, g.slab_w, 4], f32, tag="rec")
            nc.vector.reciprocal(rec[:, :nwin, :], sums[:, :nwin, :])
            aggn = fpool.tile([P, g.slab_w, HID], f16, tag="aggn")
            nc.vector.tensor_mul(
                aggn[:WIN, :nwin, :].rearrange("p w (h d) -> p w h d", h=H),
                win_ps[:, :nwin, 4:4 + HID]
                .rearrange("p w (h d) -> p w h d", h=H),
                rec[:, :nwin, :].unsqueeze(3).broadcast_to([WIN, nwin, H, HD]))
            # all windows' agg transposes into one PSUM tile, one copy out
            psTa = psG.tile([P, 4, P], f16, space="PSUM", tag="pg")
            for wl in range(nwin):
                nc.tensor.transpose(out=psTa[:, wl, :], in_=aggn[:, wl, :],
                                    identity=ident[:])
            aggT = fpool.tile([HID, g.slab_w, P], f16, tag="aggT")
            nc.scalar.activation(aggT[:, :nwin, :], psTa[:, :nwin, :],
                                 AF.Copy)
            for wl, w in enumerate(ws):
                # po reuses the window's own PSUM region (agg already read)
                po = win_ps[:, wl, 4:4 + HID]
                nc.tensor.matmul(out=po, lhsT=aggT[:, wl, 0:WIN], rhs=Wosc[:],
                                 start=True, stop=False, skip_group_check=True)
                nc.tensor.matmul(out=po, lhsT=ones_row[:, 0:WIN],
                                 rhs=beta16[:], start=False, stop=True,
                                 skip_group_check=True)
            y = fpool.tile([WIN, g.slab_w, HID], f32, tag="y")
            nc.vector.tensor_add(y[:, :nwin, :], win_ps[:, :nwin, 4:4 + HID],
                                 node_own_sb[:, w0:w0 + nwin, :])
            for wl, w in enumerate(ws):
                st6 = fpool.tile([WIN, 6], f32, tag="st6")
                nc.vector.bn_stats(st6[:], y[:, wl, :])
                nc.vector.bn_aggr(mvall[:, w, :], st6[:])
                if debug:
                    nc.sync.dma_start(dbg_y[w], y[:, wl, :])
            nc.vector.tensor_sub(
                yall[:, w0:w0 + nwin, :], y[:, :nwin, :],
                mvall[:, w0:w0 + nwin, 0:1].broadcast_to([WIN, nwin, HID]))

        nslab = len(sched["slabs"])
        stage_prep(0)
        stage_gather(0)
        stage_ohx(0)
        for s in range(nslab):
            stage_units(s)
            if s + 1 < nslab:
                stage_prep(s + 1)
                stage_gather(s + 1)
                stage_ohx(s + 1)
            stage_flush(s)

        # ---- tail: LN scale + output (keeps Ln/Exp table swap off the
        # slab loop's ACT stream, and the out DMAs off the SP queue) --------
        lnv = fpool.tile([WIN, nw], f32, tag="lnv")
        nc.scalar.activation(lnv[:], mvall[:, :, 1], AF.Ln,
                             bias=eps_col[:WIN, :])
        rstd = fpool.tile([WIN, nw], f32, tag="rstd")
        nc.scalar.activation(rstd[:], lnv[:], AF.Exp, scale=-0.5)
        QW = 14                       # windows per tail chunk
        for q0 in range(0, nw, QW):
            qn = min(QW, nw - q0)
            nc.vector.tensor_mul(
                yall[:, q0:q0 + qn, :], yall[:, q0:q0 + qn, :],
                rstd[:, q0:q0 + qn].unsqueeze(2).broadcast_to([WIN, qn, HID]))
            if not sched["skip_norm"]:
                nc.vector.tensor_mul(
                    yall[:, q0:q0 + qn, :], yall[:, q0:q0 + qn, :],
                    w_rep[:WIN, :].unsqueeze(1).broadcast_to([WIN, qn, HID]))
                nc.vector.tensor_add(
                    yall[:, q0:q0 + qn, :], yall[:, q0:q0 + qn, :],
                    b_rep[:WIN, :].unsqueeze(1).broadcast_to([WIN, qn, HID]))
            r0 = q0 * WIN
            rows = min(npc - r0, qn * WIN)
            nfull = rows // WIN
            if nfull:
                nc.sync.dma_start(
                    out[r0:r0 + nfull * WIN, :]
                    .rearrange("(w p) h -> p w h", p=WIN),
                    yall[:, q0:q0 + nfull, :])
            tail = rows - nfull * WIN
            if tail:
                nc.sync.dma_start(out[r0 + nfull * WIN:r0 + rows, :],
                                  yall[:tail, q0 + nfull, :])

    nc.compile()
    return nc


# ---------------------------------------------------------------------------
_CACHE = {}


def build_all(inputs):
    N = int(np.asarray(inputs["node_embeddings"]).shape[0])
    g = Geo(N=N, n_cores=8)
    sched, in_maps = host_prep(g, **{k: np.asarray(v) for k, v in inputs.items()})
    key = (N, sched["total_chunks"], tuple(int(x) for x in sched["caps"].ravel()),
           sched["skip_norm"], sched["n_lo"], sched["n_hi"])
    if key not in _CACHE:
        _CACHE[key] = build_program(g, sched)
    return g, sched, in_maps, _CACHE[key]


def kernel(**inputs):
    g, sched, in_maps, nc = build_all(inputs)
    from concourse.bass_utils import run_bass_kernel_spmd
    res = run_bass_kernel_spmd(nc, in_maps, core_ids=list(range(g.n_cores)))
    out = np.concatenate([res.results[c]["out"] for c in range(g.n_cores)],
                         axis=0)
    return out.astype(np.float32)


# revision 75
# speedup vs baseline: 1.0017x; 1.0017x over previous
"""GATv2 graph layer Bass kernel for TRN2 (SPMD across 8 NeuronCores).

v2 design (edge-parallel by dst range, windows of 120 dst nodes):
  - xsrc table (node_emb @ W_src, f16) built on device into DRAM (lo/hi
    halves as separate tensors so lo gathers overlap the hi build).
  - Per edge chunk (128 edges, one dst window): dma_gather xs rows; a
    combined one-hot ohx[e, 0:120]=dst, [120:128]=edge_type built by two
    broadcast is_equal ops; comb = ohxT^T @ [xdw_ext;emb] + I^T @ xs in
    PSUM (so xd gather/table is gone); ACT Prelu(0.2) -> lr f16.
  - logits = reduce32(lr * att_rep) on DVE; ex = exp(logits - 4) (shift
    cancels in softmax, keeps f16 finite); weighted = xs * ex.
  - Scatter: one matmul per chunk accumulates [sum_ex | sum(ex*xs)] into
    the window PSUM. Flush: attn-normalize, @W_out (gamma-scaled, beta
    via K=1 ones matmul), residual from transposed node cols, LayerNorm
    via bn_stats/bn_aggr + rstd = exp(-0.5*ln(var+eps)) (one ACT table).
"""
import ml_dtypes
import numpy as np
from contextlib import ExitStack
from dataclasses import dataclass

import concourse.bass as bass
import concourse.tile as tile
from concourse import bacc, mybir
from concourse.masks import make_identity

P = 128
HID = 128
H = 4
HD = 32
NET = 8
WIN = 120          # dst nodes per window (cols 120:128 select edge type)
EPS_LN = 1e-5
MAXCALL = 2048     # max idxs per dma_gather call
DEAD = -5.0        # padding marker (matches no iota column)
EXP_SHIFT = -4.0   # constant logit shift, cancels in softmax
BLK = 1024         # table build block cols


@dataclass
class Geo:
    N: int
    n_cores: int
    slab_w: int = 4

    @property
    def npc(self):
        return self.N // self.n_cores

    @property
    def nw(self):
        return (self.npc + WIN - 1) // WIN

    @property
    def nslab(self):
        return (self.nw + self.slab_w - 1) // self.slab_w

    # table geometry (n_lo/n_hi) lives in sched: the xsrc table holds only
    # the src rows actually referenced by each core's edges (compacted)


def wrap_idx(idx, cols):
    n = idx.shape[0]
    assert n % 16 == 0
    w = np.zeros((P, cols), dtype=np.int16)
    if n:
        t16 = idx.reshape(n // 16, 16).T
        for g in range(8):
            w[g * 16:(g + 1) * 16, :n // 16] = t16
    return w


def host_prep(g: Geo, node_embeddings, edge_index, edge_type, task_embedding,
              W_src, b_src, W_dst, b_dst, edge_emb, att,
              W_out, b_out, norm_w, norm_b, W_film, b_film):
    """Pure index work + tiny constant folding; all O(N*HID)/O(E*HID)
    float math runs on device."""
    src = np.asarray(edge_index[0], dtype=np.int64)
    dst = np.asarray(edge_index[1], dtype=np.int64)
    et = np.asarray(edge_type, dtype=np.int64)
    npc, nw = g.npc, g.nw

    order = np.argsort(dst, kind="stable")
    src, dst, et = src[order], dst[order], et[order]
    core_of = dst // npc

    # compact per-core src tables: only rows this core's edges reference
    used = [np.unique(src[core_of == c]) for c in range(g.n_cores)]
    n_used_max = max(len(u) for u in used)
    n_lo = ((n_used_max + 1) // 2 + BLK - 1) // BLK * BLK
    n_hi = max(BLK, ((n_used_max - min(n_used_max, n_lo)) + BLK - 1)
               // BLK * BLK)
    assert n_lo <= 32767 and n_hi <= 32767

    buckets = {}
    for c in range(g.n_cores):
        m = core_of == c
        pos = np.searchsorted(used[c], src[m])
        cd, ce = dst[m] - c * npc, et[m]
        for w in range(nw):
            wm = (cd // WIN) == w
            ws_, wd, we = pos[wm], cd[wm] - w * WIN, ce[wm]
            lo = ws_ < n_lo
            buckets[(c, w, 0)] = (ws_[lo], wd[lo], we[lo])
            buckets[(c, w, 1)] = (ws_[~lo] - n_lo, wd[~lo], we[~lo])

    caps = np.zeros((nw, 2), dtype=np.int64)
    for w in range(nw):
        for h in range(2):
            mx = max(len(buckets[(c, w, h)][0]) for c in range(g.n_cores))
            caps[w, h] = (mx + P - 1) // P
        if caps[w, 0] + caps[w, 1] == 0:
            caps[w, 0] = 1          # ensure every window PSUM gets reset

    sched_slabs = []
    total_chunks = 0
    for s in range(g.nslab):
        ws = list(range(s * g.slab_w, min((s + 1) * g.slab_w, nw)))
        chunks = []            # (wl, half, slot)
        calls = {0: [], 1: []}
        slot = 0
        for h in (0, 1):
            run = 0
            run_start = slot
            for w in ws:
                for _ in range(caps[w, h]):
                    chunks.append((w - ws[0], h, slot))
                    slot += 1
                    run += P
                    if run == MAXCALL:
                        calls[h].append((run_start, run))
                        run, run_start = 0, slot
            if run:
                calls[h].append((run_start, run))
        sched_slabs.append(dict(windows=ws, chunks=chunks, calls=calls,
                                chunk0=total_chunks))
        total_chunks += len(chunks)

    lo_cols = max(16, sum(n for sl in sched_slabs
                          for (_, n) in sl["calls"][0]) // 16)
    hi_cols = max(16, sum(n for sl in sched_slabs
                          for (_, n) in sl["calls"][1]) // 16)

    # ---- shared constants -------------------------------------------------
    node_f16 = np.asarray(node_embeddings, np.float32).astype(np.float16)
    emb_eff = (np.asarray(edge_emb, np.float64)
               + np.asarray(b_src, np.float64)[None, :]
               + np.asarray(b_dst, np.float64)[None, :]).astype(np.float16)

    consts = dict(
        W_src=np.asarray(W_src, np.float32).astype(np.float16),
        W_dst=np.asarray(W_dst, np.float32).astype(np.float16),
        W_out=np.asarray(W_out, np.float32).astype(np.float16),
        W_film=np.asarray(W_film, np.float32).astype(np.float16),
        b_film=np.asarray(b_film, np.float32).reshape(1, 2 * HID),
        b_out=np.asarray(b_out, np.float32).reshape(1, HID),
        task=np.asarray(task_embedding, np.float32).reshape(HID, 1)
            .astype(np.float16),
        emb8=emb_eff,                                        # [8, HID] f16
        att_row=np.asarray(att, np.float32).reshape(1, HID).astype(np.float16),
    )
    skip_norm = bool(np.all(np.asarray(norm_w) == 1.0)
                     and np.all(np.asarray(norm_b) == 0.0))
    if not skip_norm:
        consts["normw"] = np.asarray(norm_w, np.float32).reshape(1, HID)
        consts["normb"] = np.asarray(norm_b, np.float32).reshape(1, HID)

    # ---- per-core arrays --------------------------------------------------
    node_f32 = np.asarray(node_embeddings, np.float32)
    in_maps = []
    for c in range(g.n_cores):
        lo_l, hi_l = [], []
        dstr = np.full((P, total_chunks), DEAD, dtype=np.float16)
        etc = np.full((P, total_chunks), DEAD, dtype=np.float16)
        ci = 0
        for sl in sched_slabs:
            ws0 = sl["windows"][0]
            nth = {}
            for (wl, h, slot) in sl["chunks"]:
                w = ws0 + wl
                es, ed, ee = buckets[(c, w, h)]
                k = nth.get((wl, h), 0)
                nth[(wl, h)] = k + 1
                sl_src = np.zeros(P, dtype=np.int64)
                n = min(P, max(0, len(es) - k * P))
                if n > 0:
                    sl_src[:n] = es[k * P:k * P + n]
                    dstr[:n, ci] = ed[k * P:k * P + n]
                    etc[:n, ci] = WIN + ee[k * P:k * P + n]
                (lo_l if h == 0 else hi_l).append(sl_src)
                ci += 1
        lo_i = (np.concatenate(lo_l) if lo_l else np.zeros(0, np.int64))
        hi_i = (np.concatenate(hi_l) if hi_l else np.zeros(0, np.int64))
        assert lo_i.max(initial=0) < n_lo
        assert hi_i.max(initial=0) < n_hi

        # host-built one-hots. ohT [code, slot*128+e] feeds the comb matmul
        # lhsT directly; oh [e, slot*128+code] feeds the scatter lhsT.
        ohT = np.zeros((P, total_chunks * P), dtype=ml_dtypes.float8_e4m3fn)
        oh = np.zeros((P, total_chunks * P), dtype=ml_dtypes.float8_e4m3fn)
        pg, cg = np.meshgrid(np.arange(P), np.arange(total_chunks),
                             indexing="ij")
        for arr in (dstr, etc):
            v = arr >= 0
            code = arr[v].astype(np.int64)
            ohT[code, cg[v] * P + pg[v]] = 1.0
            oh[pg[v], cg[v] * P + code] = 1.0

        own = np.zeros((nw * WIN, HID), dtype=np.float16)
        own[:npc] = node_f32[c * npc:(c + 1) * npc].astype(np.float16)

        m = dict(consts)
        # per-core compacted src-node table (transposed), zero-padded
        nodeT = np.zeros((HID, n_lo + n_hi), dtype=np.float16)
        nodeT[:, :len(used[c])] = node_f16[used[c]].T
        m["nodeT"] = nodeT
        # transposed own nodes, 128 cols per window (last 8 zero: the emb
        # rows of xdw_ext ride in via a second accumulating matmul)
        ownT = np.zeros((HID, nw * P), dtype=np.float16)
        ot = own.T  # [HID, nw*WIN]
        for w in range(nw):
            ownT[:, w * P:w * P + WIN] = ot[:, w * WIN:(w + 1) * WIN]
        m["node_own_T"] = ownT
        m["node_own_w"] = np.ascontiguousarray(
            own.reshape(nw, WIN, HID).transpose(1, 0, 2))    # [WIN, nw, HID]
        m["lo_idx"] = wrap_idx(lo_i.astype(np.int16), lo_cols)
        m["hi_idx"] = wrap_idx(hi_i.astype(np.int16), hi_cols)
        m["ohT_in"] = ohT
        m["oh_in"] = oh
        in_maps.append(m)

    sched = dict(slabs=sched_slabs, caps=caps, total_chunks=total_chunks,
                 n_lo=n_lo, n_hi=n_hi,
                 lo_cols=lo_cols, hi_cols=hi_cols, skip_norm=skip_norm)
    return sched, in_maps


def build_program(g: Geo, sched, debug=False):
    nc = bacc.Bacc("TRN2", target_bir_lowering=False, debug=False,
                   num_devices=g.n_cores, num_swdge_queues=4)
    f16, f32 = mybir.dt.float16, mybir.dt.float32
    AF = mybir.ActivationFunctionType
    OP = mybir.AluOpType
    npc, nw = g.npc, g.nw
    total_chunks = sched["total_chunks"]
    lo_cols, hi_cols = sched["lo_cols"], sched["hi_cols"]
    n_lo = sched["n_lo"]
    n_hi = sched["n_hi"]

    def din(name, shape, dt):
        return nc.dram_tensor(name, shape, dt, kind="ExternalInput").ap()

    nodeT = din("nodeT", [HID, n_lo + n_hi], f16)
    node_own_T = din("node_own_T", [HID, nw * P], f16)
    node_own_w = din("node_own_w", [WIN, nw, HID], f16)
    W_src = din("W_src", [HID, HID], f16)
    W_dst = din("W_dst", [HID, HID], f16)
    W_out = din("W_out", [HID, HID], f16)
    W_film = din("W_film", [HID, 2 * HID], f16)
    b_film = din("b_film", [1, 2 * HID], f32)
    b_out = din("b_out", [1, HID], f32)
    task = din("task", [HID, 1], f16)
    emb8 = din("emb8", [NET, HID], f16)
    att_row = din("att_row", [1, HID], f16)
    lo_idx = din("lo_idx", [P, lo_cols], mybir.dt.int16)
    hi_idx = din("hi_idx", [P, hi_cols], mybir.dt.int16)
    f8 = mybir.dt.float8e4
    ohT_in = din("ohT_in", [P, total_chunks * P], f8)
    oh_in = din("oh_in", [P, total_chunks * P], f8)
    out = nc.dram_tensor("out", [npc, HID], f32, kind="ExternalOutput").ap()

    xsrc_lo = nc.dram_tensor("xsrc_lo", [n_lo, HID], f16).ap()
    xsrc_hi = nc.dram_tensor("xsrc_hi", [n_hi, HID], f16).ap()

    if debug:
        cmax = max(len(sl["chunks"]) for sl in sched["slabs"])
        nsl = len(sched["slabs"])
        dbg_xs = nc.dram_tensor("dbg_xs", [nsl, P, cmax, HID], f16,
                                kind="ExternalOutput").ap()
        dbg_ohx = nc.dram_tensor("dbg_ohx", [nsl, P, P, cmax], f16,
                                 kind="ExternalOutput").ap()
        dbg_lr = nc.dram_tensor("dbg_lr", [nsl, P, cmax, HID], f16,
                                kind="ExternalOutput").ap()
        dbg_rhs = nc.dram_tensor("dbg_rhs", [nsl, P, cmax, 4 + HID], f16,
                                 kind="ExternalOutput").ap()
        dbg_xdw = nc.dram_tensor("dbg_xdw", [nw, P, HID], f16,
                                 kind="ExternalOutput").ap()
        dbg_win = nc.dram_tensor("dbg_win", [nw, WIN, 4 + HID], f32,
                                 kind="ExternalOutput").ap()
        dbg_y = nc.dram_tensor("dbg_y", [nw, WIN, HID], f32,
                               kind="ExternalOutput").ap()

    with tile.TileContext(nc, trace_sim=False) as tc, ExitStack() as ctx:
        cpool = ctx.enter_context(tc.tile_pool(name="consts", bufs=1))
        bpool = ctx.enter_context(tc.tile_pool(name="build", bufs=3))
        xpool = ctx.enter_context(tc.tile_pool(name="xdw", bufs=2 * g.slab_w))
        psW = ctx.enter_context(tc.tile_pool(name="psW", bufs=1, space="PSUM"))
        spool = ctx.enter_context(tc.tile_pool(name="slab", bufs=2))
        gpool = ctx.enter_context(tc.tile_pool(name="grp", bufs=3))
        psG = ctx.enter_context(tc.tile_pool(name="psG", bufs=1, space="PSUM"))
        psC = ctx.enter_context(tc.tile_pool(name="psC", bufs=2, space="PSUM"))
        psA = ctx.enter_context(tc.tile_pool(name="psA", bufs=2, space="PSUM"))
        fpool = ctx.enter_context(tc.tile_pool(name="flush", bufs=2))
        ypool = ctx.enter_context(tc.tile_pool(name="yout", bufs=1))

        # ---- constants ----------------------------------------------------
        ident = cpool.tile([P, P], f16)
        make_identity(nc, ident[:])
        iota16 = cpool.tile([P, P], mybir.dt.int16)
        nc.gpsimd.iota(iota16[:], pattern=[[1, P]], base=0, channel_multiplier=0)
        iota = cpool.tile([P, P], f16)
        nc.vector.tensor_copy(iota[:], iota16[:])
        ones_row = cpool.tile([1, P], f16)
        nc.vector.memset(ones_row[:], 1.0)
        eps_col = cpool.tile([P, 1], f32)
        nc.vector.memset(eps_col[:], EPS_LN)
        shift_col = cpool.tile([P, 1], f32)
        nc.vector.memset(shift_col[:], EXP_SHIFT)

        Ws = cpool.tile([HID, HID], f16)
        nc.sync.dma_start(Ws[:], W_src[:])
        Wd = cpool.tile([HID, HID], f16)
        nc.sync.dma_start(Wd[:], W_dst[:])
        Wo = cpool.tile([HID, HID], f16)
        nc.sync.dma_start(Wo[:], W_out[:])
        Wf = cpool.tile([HID, 2 * HID], f16)
        nc.sync.dma_start(Wf[:], W_film[:])
        emb_sb = cpool.tile([NET, HID], f16)
        nc.sync.dma_start(emb_sb[:], emb8[:])
        att_sb = cpool.tile([1, HID], f16)
        nc.sync.dma_start(att_sb[:], att_row[:])
        task_sb = cpool.tile([HID, 1], f16)
        nc.sync.dma_start(task_sb[:], task[:])
        bfilm_sb = cpool.tile([1, 2 * HID], f32)
        nc.sync.dma_start(bfilm_sb[:], b_film[:])
        bout_sb = cpool.tile([1, HID], f32)
        nc.sync.dma_start(bout_sb[:], b_out[:])

        # residual rows, resident: node_own_sb[p, w, :] = own node w*WIN+p
        node_own_sb = cpool.tile([WIN, nw, HID], f16, tag="nodeown")
        nc.sync.dma_start(node_own_sb[:], node_own_w[:])
        # transposed own nodes, resident (feeds the per-window x_dst matmul)
        noT_all = cpool.tile([HID, nw * P], f16, tag="noT")
        nc.sync.dma_start(noT_all[:], node_own_T[:])
        # sel8[k, j] = (j == WIN + k): routes emb rows into xdw PSUM rows
        # WIN..127 via an accumulating K=8 matmul
        iotaP = cpool.tile([P, 1], mybir.dt.int16)
        nc.gpsimd.iota(iotaP[:], pattern=[[0, 1]], base=0, channel_multiplier=1)
        col8 = cpool.tile([P, 1], f32)
        nc.vector.tensor_scalar(col8[:], iotaP[:], float(WIN), None, OP.add)
        sel8 = cpool.tile([NET, P], f16)
        nc.vector.tensor_scalar(sel8[:], iota[0:NET, :], col8[0:NET, :], None,
                                OP.is_equal)

        lo_sb = cpool.tile([P, lo_cols], mybir.dt.int16, tag="loidx")
        nc.sync.dma_start(lo_sb[:], lo_idx[:])
        hi_sb = cpool.tile([P, hi_cols], mybir.dt.int16, tag="hiidx")
        nc.sync.dma_start(hi_sb[:], hi_idx[:])

        # ---- FiLM ---------------------------------------------------------
        ps_f = psW.tile([1, 2 * HID], f32, space="PSUM", tag="pw")
        nc.tensor.matmul(out=ps_f[:], lhsT=task_sb[:], rhs=Wf[:],
                         start=True, stop=True)
        film = cpool.tile([1, 2 * HID], f32)
        nc.vector.tensor_add(film[:], ps_f[:], bfilm_sb[:])
        gam_t = cpool.tile([1, HID], f32)
        nc.scalar.activation(gam_t[:], film[:, :HID], AF.Tanh)
        gam16 = cpool.tile([1, HID], f16)
        nc.vector.tensor_scalar(gam16[:], gam_t[:], 0.5, 1.0, OP.mult, OP.add)
        tmpb = cpool.tile([1, HID], f32)
        nc.vector.tensor_mul(tmpb[:], bout_sb[:], gam16[:])
        beta16 = cpool.tile([1, HID], f16)
        nc.vector.tensor_add(beta16[:], tmpb[:], film[:, HID:])
        ps_g = psW.tile([P, HID], f32, space="PSUM", tag="pw")
        nc.tensor.matmul(out=ps_g[:], lhsT=ones_row[:], rhs=gam16[:],
                         start=True, stop=True)
        gam_rep = cpool.tile([P, HID], f16)
        nc.vector.tensor_copy(gam_rep[:], ps_g[:])
        Wosc = cpool.tile([HID, HID], f16)
        nc.vector.tensor_mul(Wosc[:], Wo[:], gam_rep[:])
        ps_a = psW.tile([P, HID], f32, space="PSUM", tag="pw")
        nc.tensor.matmul(out=ps_a[:], lhsT=ones_row[:], rhs=att_sb[:],
                         start=True, stop=True)
        att_rep = cpool.tile([P, HID], f16)
        nc.vector.tensor_copy(att_rep[:], ps_a[:])

        if not sched["skip_norm"]:
            nw_dr = din("normw", [1, HID], f32)
            nb_dr = din("normb", [1, HID], f32)
            nw_sb = cpool.tile([1, HID], f32)
            nc.sync.dma_start(nw_sb[:], nw_dr[:])
            nb_sb = cpool.tile([1, HID], f32)
            nc.sync.dma_start(nb_sb[:], nb_dr[:])
            ones32 = cpool.tile([1, P], f32)
            nc.vector.memset(ones32[:], 1.0)
            ps_w = psW.tile([P, HID], f32, space="PSUM", tag="pw")
            nc.tensor.matmul(out=ps_w[:], lhsT=ones32[:], rhs=nw_sb[:],
                             start=True, stop=True)
            w_rep = cpool.tile([P, HID], f32)
            nc.vector.tensor_copy(w_rep[:], ps_w[:])
            ps_b = psW.tile([P, HID], f32, space="PSUM", tag="pw")
            nc.tensor.matmul(out=ps_b[:], lhsT=ones32[:], rhs=nb_sb[:],
                             start=True, stop=True)
            b_rep = cpool.tile([P, HID], f32)
            nc.vector.tensor_copy(b_rep[:], ps_b[:])

        # ---- xsrc table (lo half first so lo gathers can start) -----------
        for b in range((n_lo + n_hi) // BLK):
            nt = bpool.tile([HID, BLK], f16, tag="nt")
            nc.sync.dma_start(nt[:], nodeT[:, b * BLK:(b + 1) * BLK])
            xt = bpool.tile([P, 8, HID], f16, tag="xt")
            for sub in range(2):
                # rides the 2-bank comb pool (same shape/tag) so the block
                # pipeline isn't strangled by a single PSUM bank
                ps = psC.tile([P, 4, HID], f32, space="PSUM", tag="pc")
                for j in range(4):
                    nc.tensor.matmul(
                        out=ps[:, j, :],
                        lhsT=nt[:, sub * 512 + j * P: sub * 512 + (j + 1) * P],
                        rhs=Ws[:], start=True, stop=True,
                        skip_group_check=True)
                if sub == 0:
                    nc.scalar.activation(xt[:, :4, :], ps[:], AF.Copy)
                else:
                    nc.vector.tensor_copy(xt[:, 4:, :], ps[:])
            r0 = b * BLK
            tab, base = (xsrc_lo, 0) if r0 < n_lo else (xsrc_hi, n_lo)
            # table writes ride the ACT hwdge queue; SP carries the reads
            nc.scalar.dma_start(
                tab[r0 - base:r0 - base + BLK, :]
                .rearrange("(s p) h -> p s h", p=P), xt[:])

        # ---- slabs --------------------------------------------------------
        yall = ypool.tile([WIN, nw, HID], f32, tag="yall")
        mvall = ypool.tile([WIN, nw, 2], f32, tag="mvall")
        off16 = {0: 0, 1: 0}
        qn = [0]

        def nextq():
            qn[0] = (qn[0] + 1) % 4
            return qn[0]

        UNIT = 8
        slab_state = {}

        def stage_prep(s):
            """xdw_ext tiles (rows 0:WIN = x_dst via W_dst, WIN:128 = emb)."""
            sl = sched["slabs"][s]
            xdw_t = []
            for wl, w in enumerate(sl["windows"]):
                px = psW.tile([P, HID], f32, space="PSUM", tag="pw")
                nc.tensor.matmul(out=px[:],
                                 lhsT=noT_all[:, w * P:(w + 1) * P],
                                 rhs=Wd[:], start=True, stop=False,
                                 skip_group_check=True)
                nc.tensor.matmul(out=px[:], lhsT=sel8[:], rhs=emb_sb[:],
                                 start=False, stop=True,
                                 skip_group_check=True)
                xdw = xpool.tile([P, HID], f16, tag="xdw",
                                 name=f"xdw{s}_{wl}")
                nc.scalar.activation(xdw[:], px[:], AF.Copy)
                xdw_t.append(xdw)
            C = len(sl["chunks"])
            win_ps = psA.tile([WIN, g.slab_w, 256], f32, space="PSUM",
                              tag="win", name=f"win{s}")
            nc.scalar.memzero(win_ps[:])
            c0 = sl["chunk0"]
            ohxT = spool.tile([P, C, P], mybir.dt.float8e4, tag="ohT",
                              name=f"ohT{s}")
            nc.sync.dma_start(ohxT[:],
                              ohT_in[:, c0 * P:(c0 + C) * P]
                              .rearrange("p (c e) -> p c e", e=P))
            slab_state[s] = dict(
                xdw_t=xdw_t,
                xs_t=spool.tile([P, C, HID], f16, tag="xs", name=f"xs{s}"),
                ohx=spool.tile([P, C, P], mybir.dt.float8e4, tag="ohx",
                               name=f"ohx{s}"),
                win_ps=win_ps, ohxT=ohxT,
            )

        def stage_gather(s):
            sl = sched["slabs"][s]
            xs_t = slab_state[s]["xs_t"]
            for h, (idx_sb, tab, tabn) in enumerate(
                    ((lo_sb, xsrc_lo, n_lo), (hi_sb, xsrc_hi, n_hi))):
                for (slot_off, n) in sl["calls"][h]:
                    if n == 0:
                        continue
                    nc.gpsimd.dma_gather(
                        out_ap=xs_t[:, slot_off:slot_off + n // P, :],
                        in_ap=tab[0:tabn, :],
                        idxs_ap=idx_sb[:, off16[h]:off16[h] + n // 16],
                        num_idxs=n, num_idxs_reg=n, elem_size=HID,
                        single_packet=(n <= 1024), queue_num=nextq(),
                    )
                    off16[h] += n // 16

        def stage_ohx(s):
            """Edge-major one-hot (scatter lhsT), host-built, via DMA."""
            sl = sched["slabs"][s]
            C = len(sl["chunks"])
            c0 = sl["chunk0"]
            ohx = slab_state[s]["ohx"]
            nc.sync.dma_start(ohx[:],
                              oh_in[:, c0 * P:(c0 + C) * P]
                              .rearrange("p (c e) -> p c e", e=P))

        def stage_units(s):
            sl = sched["slabs"][s]
            chunks = sl["chunks"]
            C = len(chunks)
            st = slab_state[s]
            xs_t, ohx, xdw_t = st["xs_t"], st["ohx"], st["xdw_t"]
            lr = spool.tile([P, C, HID], f16, tag="lr", name=f"lr{s}")
            rhs_t = spool.tile([P, C, 4 + HID], f16, tag="rhs", name=f"rhs{s}")
            logits = gpool.tile([P, C, H], f16, tag="logit", name=f"lg{s}")
            st["rhs_t"] = rhs_t
            st["lr"] = lr

            ohxT = st["ohxT"]

            def unit_pe(u0, un):
                # comb psum + Prelu, per 4-chunk psum group (ohxT host-built)
                for g0 in range(u0, u0 + un, 4):
                    gn = min(4, u0 + un - g0)
                    psc = psC.tile([P, 4, HID], f32, space="PSUM", tag="pc")
                    for k in range(gn):
                        wl = chunks[g0 + k][0]
                        nc.tensor.matmul(out=psc[:, k, :],
                                         lhsT=ohxT[:, g0 + k, :],
                                         rhs=xdw_t[wl][:], start=True,
                                         stop=False, skip_group_check=True)
                        nc.tensor.matmul(out=psc[:, k, :], lhsT=ident[:],
                                         rhs=xs_t[:, g0 + k, :], start=False,
                                         stop=True, skip_group_check=True)
                    nc.scalar.activation(lr[:, g0:g0 + gn, :], psc[:, :gn, :],
                                         AF.Prelu, alpha=0.2)

            def unit_post(u0, un):
                # logits / softmax-numerator / weighted messages
                lr4 = lr[:, u0:u0 + un, :].rearrange(
                    "p c (h d) -> p c h d", h=H)
                wgt4 = rhs_t[:, u0:u0 + un, 4:4 + HID].rearrange(
                    "p c (h d) -> p c h d", h=H)
                nc.vector.tensor_mul(
                    wgt4, lr4,
                    att_rep[:].rearrange("p (h d) -> p h d", h=H).unsqueeze(1)
                    .broadcast_to([P, un, H, HD]))
                # tree-fold before the (slow-mode) reduce: adds run at 2x
                nc.vector.tensor_add(wgt4[:, :, :, 0:16], wgt4[:, :, :, 0:16],
                                     wgt4[:, :, :, 16:32])
                nc.vector.tensor_add(wgt4[:, :, :, 0:8], wgt4[:, :, :, 0:8],
                                     wgt4[:, :, :, 8:16])
                with nc.allow_low_precision(reason="logits |x|<12, f16 ok"):
                    nc.vector.tensor_reduce(
                        logits[:, u0:u0 + un, :], wgt4[:, :, :, 0:8],
                        axis=mybir.AxisListType.X, op=OP.add)
                # exp with broadcast INPUT: ex_rep comes out pre-expanded to
                # 32 cols/head so the weighted mul keeps packed operands
                ex_rep = gpool.tile([P, UNIT, HID], f16, tag="exrep")
                nc.scalar.activation(
                    ex_rep[:, :un, :],
                    logits[:, u0:u0 + un, :].unsqueeze(3)
                    .broadcast_to([P, un, H, HD]),
                    AF.Exp, bias=shift_col[:])
                nc.vector.tensor_copy(
                    rhs_t[:, u0:u0 + un, 0:4],
                    ex_rep[:, :un, :].rearrange(
                        "p c (h d) -> p c h d", h=H)[:, :, :, 0])
                nc.vector.tensor_mul(
                    rhs_t[:, u0:u0 + un, 4:4 + HID],
                    xs_t[:, u0:u0 + un, :], ex_rep[:, :un, :])

            # scatter: the bank was memset in stage_prep; all matmuls
            # accumulate (start=False), stop=True on each window's last
            # chunk. Interleaving accumulate-mode writes in one bank is
            # safe (unlike interleaved start=True groups, which corrupt).
            win_ps = st["win_ps"]
            last_slot = {}
            for (wl, h, slot) in chunks:
                last_slot[wl] = slot

            def unit_scatter(u0, un):
                for slot in range(u0, u0 + un):
                    wl = chunks[slot][0]
                    nc.tensor.matmul(out=win_ps[:, wl, 0:4 + HID],
                                     lhsT=ohx[:, slot, 0:WIN],
                                     rhs=rhs_t[:, slot, :],
                                     start=False,
                                     stop=(last_slot[wl] == slot),
                                     skip_group_check=True)

            units = [(u0, min(UNIT, C - u0)) for u0 in range(0, C, UNIT)]
            pend = []
            for (u0, un) in units:
                unit_pe(u0, un)
                if pend:
                    unit_post(*pend[-1])
                if len(pend) >= 2:
                    unit_scatter(*pend[-2])  # two-unit lag: zero PE stall
                pend.append((u0, un))
            unit_post(*pend[-1])
            for (u0, un) in pend[-2:]:
                unit_scatter(u0, un)

        def stage_flush(s):
            sl = sched["slabs"][s]
            ws = sl["windows"]
            nwin = len(ws)
            st = slab_state[s]
            win_ps = st["win_ps"]
            w0 = ws[0]
            if debug:
                C = len(sl["chunks"])
                nc.sync.dma_start(dbg_xs[s, :, :C, :], st["xs_t"][:])
                nc.sync.dma_start(dbg_ohx[s, :, :, :C],
                                  st["ohx"][:].rearrange("p c e -> p e c"))
                nc.sync.dma_start(dbg_lr[s, :, :C, :], st["lr"][:])
                nc.sync.dma_start(dbg_rhs[s, :, :C, :], st["rhs_t"][:])
                for wl, w in enumerate(ws):
                    nc.sync.dma_start(dbg_xdw[w], st["xdw_t"][wl][:])
                    dbw = fpool.tile([WIN, 4 + HID], f32, tag="dbw")
                    nc.vector.tensor_copy(dbw[:], win_ps[:, wl, :])
                    nc.sync.dma_start(dbg_win[w], dbw[:])
            # attention normalization, batched across the slab's windows
            sums = fpool.tile([WIN, g.slab_w, 4], f32, tag="sums")
            nc.vector.tensor_scalar(sums[:, :nwin, :], win_ps[:, :nwin, 0:4],
                                    1e-12, None, OP.max)
            rec = fpool.tile([WIN, g.slab_w, 4], f32, tag="rec")
            nc.vector.reciprocal(rec[:, :nwin, :], sums[:, :nwin, :])
            aggn = fpool.tile([P, g.slab_w, HID], f16, tag="aggn")
            nc.vector.tensor_mul(
                aggn[:WIN, :nwin, :].rearrange("p w (h d) -> p w h d", h=H),
                win_ps[:, :nwin, 4:4 + HID]
                .rearrange("p w (h d) -> p w h d", h=H),
                rec[:, :nwin, :].unsqueeze(3).broadcast_to([WIN, nwin, H, HD]))
            # all windows' agg transposes into one PSUM tile, one copy out
            psTa = psG.tile([P, 4, P], f16, space="PSUM", tag="pg")
            for wl in range(nwin):
                nc.tensor.transpose(out=psTa[:, wl, :], in_=aggn[:, wl, :],
                                    identity=ident[:])
            aggT = fpool.tile([HID, g.slab_w, P], f16, tag="aggT")
            nc.scalar.activation(aggT[:, :nwin, :], psTa[:, :nwin, :],
                                 AF.Copy)
            for wl, w in enumerate(ws):
                # po reuses the window's own PSUM region (agg already read)
                po = win_ps[:, wl, 4:4 + HID]
                nc.tensor.matmul(out=po, lhsT=aggT[:, wl, 0:WIN], rhs=Wosc[:],
                                 start=True, stop=False, skip_group_check=True)
                nc.tensor.matmul(out=po, lhsT=ones_row[:, 0:WIN],
                                 rhs=beta16[:], start=False, stop=True,
                                 skip_group_check=True)
            y = fpool.tile([WIN, g.slab_w, HID], f32, tag="y")
            nc.vector.tensor_add(y[:, :nwin, :], win_ps[:, :nwin, 4:4 + HID],
                                 node_own_sb[:, w0:w0 + nwin, :])
            for wl, w in enumerate(ws):
                st6 = fpool.tile([WIN, 6], f32, tag="st6")
                nc.vector.bn_stats(st6[:], y[:, wl, :])
                nc.vector.bn_aggr(mvall[:, w, :], st6[:])
                if debug:
                    nc.sync.dma_start(dbg_y[w], y[:, wl, :])
            nc.vector.tensor_sub(
                yall[:, w0:w0 + nwin, :], y[:, :nwin, :],
                mvall[:, w0:w0 + nwin, 0:1].broadcast_to([WIN, nwin, HID]))

        nslab = len(sched["slabs"])
        stage_prep(0)
        stage_gather(0)
        stage_ohx(0)
        for s in range(nslab):
            stage_units(s)
            if s + 1 < nslab:
                stage_prep(s + 1)
                stage_gather(s + 1)
                stage_ohx(s + 1)
            stage_flush(s)

        # ---- tail: LN scale + output (keeps Ln/Exp table swap off the
        # slab loop's ACT stream, and the out DMAs off the SP queue) --------
        lnv = fpool.tile([WIN, nw], f32, tag="lnv")
        nc.scalar.activation(lnv[:], mvall[:, :, 1], AF.Ln,
                             bias=eps_col[:WIN, :])
        rstd = fpool.tile([WIN, nw], f32, tag="rstd")
        nc.scalar.activation(rstd[:], lnv[:], AF.Exp, scale=-0.5)
        QW = 14                       # windows per tail chunk
        for q0 in range(0, nw, QW):
            qn = min(QW, nw - q0)
            nc.vector.tensor_mul(
                yall[:, q0:q0 + qn, :], yall[:, q0:q0 + qn, :],
                rstd[:, q0:q0 + qn].unsqueeze(2).broadcast_to([WIN, qn, HID]))
            if not sched["skip_norm"]:
                nc.vector.tensor_mul(
                    yall[:, q0:q0 + qn, :], yall[:, q0:q0 + qn, :],
                    w_rep[:WIN, :].unsqueeze(1).broadcast_to([WIN, qn, HID]))
                nc.vector.tensor_add(
                    yall[:, q0:q0 + qn, :], yall[:, q0:q0 + qn, :],
                    b_rep[:WIN, :].unsqueeze(1).broadcast_to([WIN, qn, HID]))
            r0 = q0 * WIN
            rows = min(npc - r0, qn * WIN)
            nfull = rows // WIN
            if nfull:
                nc.sync.dma_start(
                    out[r0:r0 + nfull * WIN, :]
                    .rearrange("(w p) h -> p w h", p=WIN),
                    yall[:, q0:q0 + nfull, :])
            tail = rows - nfull * WIN
            if tail:
                nc.sync.dma_start(out[r0 + nfull * WIN:r0 + rows, :],
                                  yall[:tail, q0 + nfull, :])

    nc.compile()
    return nc


# ---------------------------------------------------------------------------
_CACHE = {}


def build_all(inputs):
    N = int(np.asarray(inputs["node_embeddings"]).shape[0])
    g = Geo(N=N, n_cores=8)
    sched, in_maps = host_prep(g, **{k: np.asarray(v) for k, v in inputs.items()})
    key = (N, sched["total_chunks"], tuple(int(x) for x in sched["caps"].ravel()),
           sched["skip_norm"], sched["n_lo"], sched["n_hi"])
    if key not in _CACHE:
        _CACHE[key] = build_program(g, sched)
    return g, sched, in_maps, _CACHE[key]


def kernel(**inputs):
    g, sched, in_maps, nc = build_all(inputs)
    from concourse.bass_utils import run_bass_kernel_spmd
    res = run_bass_kernel_spmd(nc, in_maps, core_ids=list(range(g.n_cores)))
    out = np.concatenate([res.results[c]["out"] for c in range(g.n_cores)],
                         axis=0)
    return out.astype(np.float32)


# revision 76
# speedup vs baseline: 1.0083x; 1.0066x over previous
"""GATv2 graph layer Bass kernel for TRN2 (SPMD across 8 NeuronCores).

v2 design (edge-parallel by dst range, windows of 120 dst nodes):
  - xsrc table (node_emb @ W_src, f16) built on device into DRAM (lo/hi
    halves as separate tensors so lo gathers overlap the hi build).
  - Per edge chunk (128 edges, one dst window): dma_gather xs rows; a
    combined one-hot ohx[e, 0:120]=dst, [120:128]=edge_type built by two
    broadcast is_equal ops; comb = ohxT^T @ [xdw_ext;emb] + I^T @ xs in
    PSUM (so xd gather/table is gone); ACT Prelu(0.2) -> lr f16.
  - logits = reduce32(lr * att_rep) on DVE; ex = exp(logits - 4) (shift
    cancels in softmax, keeps f16 finite); weighted = xs * ex.
  - Scatter: one matmul per chunk accumulates [sum_ex | sum(ex*xs)] into
    the window PSUM. Flush: attn-normalize, @W_out (gamma-scaled, beta
    via K=1 ones matmul), residual from transposed node cols, LayerNorm
    via bn_stats/bn_aggr + rstd = exp(-0.5*ln(var+eps)) (one ACT table).
"""
import ml_dtypes
import numpy as np
from contextlib import ExitStack
from dataclasses import dataclass

import concourse.bass as bass
import concourse.tile as tile
from concourse import bacc, mybir
from concourse.masks import make_identity

P = 128
HID = 128
H = 4
HD = 32
NET = 8
WIN = 120          # dst nodes per window (cols 120:128 select edge type)
EPS_LN = 1e-5
MAXCALL = 1024     # max idxs per dma_gather call
DEAD = -5.0        # padding marker (matches no iota column)
EXP_SHIFT = -4.0   # constant logit shift, cancels in softmax
BLK = 1024         # table build block cols


@dataclass
class Geo:
    N: int
    n_cores: int
    slab_w: int = 4

    @property
    def npc(self):
        return self.N // self.n_cores

    @property
    def nw(self):
        return (self.npc + WIN - 1) // WIN

    @property
    def nslab(self):
        return (self.nw + self.slab_w - 1) // self.slab_w

    # table geometry (n_lo/n_hi) lives in sched: the xsrc table holds only
    # the src rows actually referenced by each core's edges (compacted)


def wrap_idx(idx, cols):
    n = idx.shape[0]
    assert n % 16 == 0
    w = np.zeros((P, cols), dtype=np.int16)
    if n:
        t16 = idx.reshape(n // 16, 16).T
        for g in range(8):
            w[g * 16:(g + 1) * 16, :n // 16] = t16
    return w


def host_prep(g: Geo, node_embeddings, edge_index, edge_type, task_embedding,
              W_src, b_src, W_dst, b_dst, edge_emb, att,
              W_out, b_out, norm_w, norm_b, W_film, b_film):
    """Pure index work + tiny constant folding; all O(N*HID)/O(E*HID)
    float math runs on device."""
    src = np.asarray(edge_index[0], dtype=np.int64)
    dst = np.asarray(edge_index[1], dtype=np.int64)
    et = np.asarray(edge_type, dtype=np.int64)
    npc, nw = g.npc, g.nw

    order = np.argsort(dst, kind="stable")
    src, dst, et = src[order], dst[order], et[order]
    core_of = dst // npc

    # compact per-core src tables: only rows this core's edges reference
    used = [np.unique(src[core_of == c]) for c in range(g.n_cores)]
    n_used_max = max(len(u) for u in used)
    n_lo = ((n_used_max + 1) // 2 + BLK - 1) // BLK * BLK
    n_hi = max(BLK, ((n_used_max - min(n_used_max, n_lo)) + BLK - 1)
               // BLK * BLK)
    assert n_lo <= 32767 and n_hi <= 32767

    buckets = {}
    for c in range(g.n_cores):
        m = core_of == c
        pos = np.searchsorted(used[c], src[m])
        cd, ce = dst[m] - c * npc, et[m]
        for w in range(nw):
            wm = (cd // WIN) == w
            ws_, wd, we = pos[wm], cd[wm] - w * WIN, ce[wm]
            lo = ws_ < n_lo
            buckets[(c, w, 0)] = (ws_[lo], wd[lo], we[lo])
            buckets[(c, w, 1)] = (ws_[~lo] - n_lo, wd[~lo], we[~lo])

    caps = np.zeros((nw, 2), dtype=np.int64)
    for w in range(nw):
        for h in range(2):
            mx = max(len(buckets[(c, w, h)][0]) for c in range(g.n_cores))
            caps[w, h] = (mx + P - 1) // P
        if caps[w, 0] + caps[w, 1] == 0:
            caps[w, 0] = 1          # ensure every window PSUM gets reset

    sched_slabs = []
    total_chunks = 0
    for s in range(g.nslab):
        ws = list(range(s * g.slab_w, min((s + 1) * g.slab_w, nw)))
        chunks = []            # (wl, half, slot)
        calls = {0: [], 1: []}
        slot = 0
        for h in (0, 1):
            run = 0
            run_start = slot
            for w in ws:
                for _ in range(caps[w, h]):
                    chunks.append((w - ws[0], h, slot))
                    slot += 1
                    run += P
                    if run == MAXCALL:
                        calls[h].append((run_start, run))
                        run, run_start = 0, slot
            if run:
                calls[h].append((run_start, run))
        sched_slabs.append(dict(windows=ws, chunks=chunks, calls=calls,
                                chunk0=total_chunks))
        total_chunks += len(chunks)

    lo_cols = max(16, sum(n for sl in sched_slabs
                          for (_, n) in sl["calls"][0]) // 16)
    hi_cols = max(16, sum(n for sl in sched_slabs
                          for (_, n) in sl["calls"][1]) // 16)

    # ---- shared constants -------------------------------------------------
    node_f16 = np.asarray(node_embeddings, np.float32).astype(np.float16)
    emb_eff = (np.asarray(edge_emb, np.float64)
               + np.asarray(b_src, np.float64)[None, :]
               + np.asarray(b_dst, np.float64)[None, :]).astype(np.float16)

    consts = dict(
        W_src=np.asarray(W_src, np.float32).astype(np.float16),
        W_dst=np.asarray(W_dst, np.float32).astype(np.float16),
        W_out=np.asarray(W_out, np.float32).astype(np.float16),
        W_film=np.asarray(W_film, np.float32).astype(np.float16),
        b_film=np.asarray(b_film, np.float32).reshape(1, 2 * HID),
        b_out=np.asarray(b_out, np.float32).reshape(1, HID),
        task=np.asarray(task_embedding, np.float32).reshape(HID, 1)
            .astype(np.float16),
        emb8=emb_eff,                                        # [8, HID] f16
        att_row=np.asarray(att, np.float32).reshape(1, HID).astype(np.float16),
    )
    skip_norm = bool(np.all(np.asarray(norm_w) == 1.0)
                     and np.all(np.asarray(norm_b) == 0.0))
    if not skip_norm:
        consts["normw"] = np.asarray(norm_w, np.float32).reshape(1, HID)
        consts["normb"] = np.asarray(norm_b, np.float32).reshape(1, HID)

    # ---- per-core arrays --------------------------------------------------
    node_f32 = np.asarray(node_embeddings, np.float32)
    in_maps = []
    for c in range(g.n_cores):
        lo_l, hi_l = [], []
        dstr = np.full((P, total_chunks), DEAD, dtype=np.float16)
        etc = np.full((P, total_chunks), DEAD, dtype=np.float16)
        ci = 0
        for sl in sched_slabs:
            ws0 = sl["windows"][0]
            nth = {}
            for (wl, h, slot) in sl["chunks"]:
                w = ws0 + wl
                es, ed, ee = buckets[(c, w, h)]
                k = nth.get((wl, h), 0)
                nth[(wl, h)] = k + 1
                sl_src = np.zeros(P, dtype=np.int64)
                n = min(P, max(0, len(es) - k * P))
                if n > 0:
                    sl_src[:n] = es[k * P:k * P + n]
                    dstr[:n, ci] = ed[k * P:k * P + n]
                    etc[:n, ci] = WIN + ee[k * P:k * P + n]
                (lo_l if h == 0 else hi_l).append(sl_src)
                ci += 1
        lo_i = (np.concatenate(lo_l) if lo_l else np.zeros(0, np.int64))
        hi_i = (np.concatenate(hi_l) if hi_l else np.zeros(0, np.int64))
        assert lo_i.max(initial=0) < n_lo
        assert hi_i.max(initial=0) < n_hi

        # host-built one-hots. ohT [code, slot*128+e] feeds the comb matmul
        # lhsT directly; oh [e, slot*128+code] feeds the scatter lhsT.
        ohT = np.zeros((P, total_chunks * P), dtype=ml_dtypes.float8_e4m3fn)
        oh = np.zeros((P, total_chunks * P), dtype=ml_dtypes.float8_e4m3fn)
        pg, cg = np.meshgrid(np.arange(P), np.arange(total_chunks),
                             indexing="ij")
        for arr in (dstr, etc):
            v = arr >= 0
            code = arr[v].astype(np.int64)
            ohT[code, cg[v] * P + pg[v]] = 1.0
            oh[pg[v], cg[v] * P + code] = 1.0

        own = np.zeros((nw * WIN, HID), dtype=np.float16)
        own[:npc] = node_f32[c * npc:(c + 1) * npc].astype(np.float16)

        m = dict(consts)
        # per-core compacted src-node table (transposed), zero-padded
        nodeT = np.zeros((HID, n_lo + n_hi), dtype=np.float16)
        nodeT[:, :len(used[c])] = node_f16[used[c]].T
        m["nodeT"] = nodeT
        # transposed own nodes, 128 cols per window (last 8 zero: the emb
        # rows of xdw_ext ride in via a second accumulating matmul)
        ownT = np.zeros((HID, nw * P), dtype=np.float16)
        ot = own.T  # [HID, nw*WIN]
        for w in range(nw):
            ownT[:, w * P:w * P + WIN] = ot[:, w * WIN:(w + 1) * WIN]
        m["node_own_T"] = ownT
        m["node_own_w"] = np.ascontiguousarray(
            own.reshape(nw, WIN, HID).transpose(1, 0, 2))    # [WIN, nw, HID]
        m["lo_idx"] = wrap_idx(lo_i.astype(np.int16), lo_cols)
        m["hi_idx"] = wrap_idx(hi_i.astype(np.int16), hi_cols)
        m["ohT_in"] = ohT
        m["oh_in"] = oh
        in_maps.append(m)

    sched = dict(slabs=sched_slabs, caps=caps, total_chunks=total_chunks,
                 n_lo=n_lo, n_hi=n_hi,
                 lo_cols=lo_cols, hi_cols=hi_cols, skip_norm=skip_norm)
    return sched, in_maps


def build_program(g: Geo, sched, debug=False):
    nc = bacc.Bacc("TRN2", target_bir_lowering=False, debug=False,
                   num_devices=g.n_cores, num_swdge_queues=4)
    f16, f32 = mybir.dt.float16, mybir.dt.float32
    AF = mybir.ActivationFunctionType
    OP = mybir.AluOpType
    npc, nw = g.npc, g.nw
    total_chunks = sched["total_chunks"]
    lo_cols, hi_cols = sched["lo_cols"], sched["hi_cols"]
    n_lo = sched["n_lo"]
    n_hi = sched["n_hi"]

    def din(name, shape, dt):
        return nc.dram_tensor(name, shape, dt, kind="ExternalInput").ap()

    nodeT = din("nodeT", [HID, n_lo + n_hi], f16)
    node_own_T = din("node_own_T", [HID, nw * P], f16)
    node_own_w = din("node_own_w", [WIN, nw, HID], f16)
    W_src = din("W_src", [HID, HID], f16)
    W_dst = din("W_dst", [HID, HID], f16)
    W_out = din("W_out", [HID, HID], f16)
    W_film = din("W_film", [HID, 2 * HID], f16)
    b_film = din("b_film", [1, 2 * HID], f32)
    b_out = din("b_out", [1, HID], f32)
    task = din("task", [HID, 1], f16)
    emb8 = din("emb8", [NET, HID], f16)
    att_row = din("att_row", [1, HID], f16)
    lo_idx = din("lo_idx", [P, lo_cols], mybir.dt.int16)
    hi_idx = din("hi_idx", [P, hi_cols], mybir.dt.int16)
    f8 = mybir.dt.float8e4
    ohT_in = din("ohT_in", [P, total_chunks * P], f8)
    oh_in = din("oh_in", [P, total_chunks * P], f8)
    out = nc.dram_tensor("out", [npc, HID], f32, kind="ExternalOutput").ap()

    xsrc_lo = nc.dram_tensor("xsrc_lo", [n_lo, HID], f16).ap()
    xsrc_hi = nc.dram_tensor("xsrc_hi", [n_hi, HID], f16).ap()

    if debug:
        cmax = max(len(sl["chunks"]) for sl in sched["slabs"])
        nsl = len(sched["slabs"])
        dbg_xs = nc.dram_tensor("dbg_xs", [nsl, P, cmax, HID], f16,
                                kind="ExternalOutput").ap()
        dbg_ohx = nc.dram_tensor("dbg_ohx", [nsl, P, P, cmax], f16,
                                 kind="ExternalOutput").ap()
        dbg_lr = nc.dram_tensor("dbg_lr", [nsl, P, cmax, HID], f16,
                                kind="ExternalOutput").ap()
        dbg_rhs = nc.dram_tensor("dbg_rhs", [nsl, P, cmax, 4 + HID], f16,
                                 kind="ExternalOutput").ap()
        dbg_xdw = nc.dram_tensor("dbg_xdw", [nw, P, HID], f16,
                                 kind="ExternalOutput").ap()
        dbg_win = nc.dram_tensor("dbg_win", [nw, WIN, 4 + HID], f32,
                                 kind="ExternalOutput").ap()
        dbg_y = nc.dram_tensor("dbg_y", [nw, WIN, HID], f32,
                               kind="ExternalOutput").ap()

    with tile.TileContext(nc, trace_sim=False) as tc, ExitStack() as ctx:
        cpool = ctx.enter_context(tc.tile_pool(name="consts", bufs=1))
        bpool = ctx.enter_context(tc.tile_pool(name="build", bufs=3))
        xpool = ctx.enter_context(tc.tile_pool(name="xdw", bufs=2 * g.slab_w))
        psW = ctx.enter_context(tc.tile_pool(name="psW", bufs=1, space="PSUM"))
        spool = ctx.enter_context(tc.tile_pool(name="slab", bufs=2))
        gpool = ctx.enter_context(tc.tile_pool(name="grp", bufs=3))
        psG = ctx.enter_context(tc.tile_pool(name="psG", bufs=1, space="PSUM"))
        psC = ctx.enter_context(tc.tile_pool(name="psC", bufs=2, space="PSUM"))
        psA = ctx.enter_context(tc.tile_pool(name="psA", bufs=2, space="PSUM"))
        fpool = ctx.enter_context(tc.tile_pool(name="flush", bufs=2))
        ypool = ctx.enter_context(tc.tile_pool(name="yout", bufs=1))

        # ---- constants ----------------------------------------------------
        ident = cpool.tile([P, P], f16)
        make_identity(nc, ident[:])
        iota16 = cpool.tile([P, P], mybir.dt.int16)
        nc.gpsimd.iota(iota16[:], pattern=[[1, P]], base=0, channel_multiplier=0)
        iota = cpool.tile([P, P], f16)
        nc.vector.tensor_copy(iota[:], iota16[:])
        ones_row = cpool.tile([1, P], f16)
        nc.vector.memset(ones_row[:], 1.0)
        eps_col = cpool.tile([P, 1], f32)
        nc.vector.memset(eps_col[:], EPS_LN)
        shift_col = cpool.tile([P, 1], f32)
        nc.vector.memset(shift_col[:], EXP_SHIFT)

        Ws = cpool.tile([HID, HID], f16)
        nc.sync.dma_start(Ws[:], W_src[:])
        Wd = cpool.tile([HID, HID], f16)
        nc.sync.dma_start(Wd[:], W_dst[:])
        Wo = cpool.tile([HID, HID], f16)
        nc.sync.dma_start(Wo[:], W_out[:])
        Wf = cpool.tile([HID, 2 * HID], f16)
        nc.sync.dma_start(Wf[:], W_film[:])
        emb_sb = cpool.tile([NET, HID], f16)
        nc.sync.dma_start(emb_sb[:], emb8[:])
        att_sb = cpool.tile([1, HID], f16)
        nc.sync.dma_start(att_sb[:], att_row[:])
        task_sb = cpool.tile([HID, 1], f16)
        nc.sync.dma_start(task_sb[:], task[:])
        bfilm_sb = cpool.tile([1, 2 * HID], f32)
        nc.sync.dma_start(bfilm_sb[:], b_film[:])
        bout_sb = cpool.tile([1, HID], f32)
        nc.sync.dma_start(bout_sb[:], b_out[:])

        # residual rows, resident: node_own_sb[p, w, :] = own node w*WIN+p
        node_own_sb = cpool.tile([WIN, nw, HID], f16, tag="nodeown")
        nc.sync.dma_start(node_own_sb[:], node_own_w[:])
        # transposed own nodes, resident (feeds the per-window x_dst matmul)
        noT_all = cpool.tile([HID, nw * P], f16, tag="noT")
        nc.sync.dma_start(noT_all[:], node_own_T[:])
        # sel8[k, j] = (j == WIN + k): routes emb rows into xdw PSUM rows
        # WIN..127 via an accumulating K=8 matmul
        iotaP = cpool.tile([P, 1], mybir.dt.int16)
        nc.gpsimd.iota(iotaP[:], pattern=[[0, 1]], base=0, channel_multiplier=1)
        col8 = cpool.tile([P, 1], f32)
        nc.vector.tensor_scalar(col8[:], iotaP[:], float(WIN), None, OP.add)
        sel8 = cpool.tile([NET, P], f16)
        nc.vector.tensor_scalar(sel8[:], iota[0:NET, :], col8[0:NET, :], None,
                                OP.is_equal)

        lo_sb = cpool.tile([P, lo_cols], mybir.dt.int16, tag="loidx")
        nc.sync.dma_start(lo_sb[:], lo_idx[:])
        hi_sb = cpool.tile([P, hi_cols], mybir.dt.int16, tag="hiidx")
        nc.sync.dma_start(hi_sb[:], hi_idx[:])

        # ---- FiLM ---------------------------------------------------------
        ps_f = psW.tile([1, 2 * HID], f32, space="PSUM", tag="pw")
        nc.tensor.matmul(out=ps_f[:], lhsT=task_sb[:], rhs=Wf[:],
                         start=True, stop=True)
        film = cpool.tile([1, 2 * HID], f32)
        nc.vector.tensor_add(film[:], ps_f[:], bfilm_sb[:])
        gam_t = cpool.tile([1, HID], f32)
        nc.scalar.activation(gam_t[:], film[:, :HID], AF.Tanh)
        gam16 = cpool.tile([1, HID], f16)
        nc.vector.tensor_scalar(gam16[:], gam_t[:], 0.5, 1.0, OP.mult, OP.add)
        tmpb = cpool.tile([1, HID], f32)
        nc.vector.tensor_mul(tmpb[:], bout_sb[:], gam16[:])
        beta16 = cpool.tile([1, HID], f16)
        nc.vector.tensor_add(beta16[:], tmpb[:], film[:, HID:])
        ps_g = psW.tile([P, HID], f32, space="PSUM", tag="pw")
        nc.tensor.matmul(out=ps_g[:], lhsT=ones_row[:], rhs=gam16[:],
                         start=True, stop=True)
        gam_rep = cpool.tile([P, HID], f16)
        nc.vector.tensor_copy(gam_rep[:], ps_g[:])
        Wosc = cpool.tile([HID, HID], f16)
        nc.vector.tensor_mul(Wosc[:], Wo[:], gam_rep[:])
        ps_a = psW.tile([P, HID], f32, space="PSUM", tag="pw")
        nc.tensor.matmul(out=ps_a[:], lhsT=ones_row[:], rhs=att_sb[:],
                         start=True, stop=True)
        att_rep = cpool.tile([P, HID], f16)
        nc.vector.tensor_copy(att_rep[:], ps_a[:])

        if not sched["skip_norm"]:
            nw_dr = din("normw", [1, HID], f32)
            nb_dr = din("normb", [1, HID], f32)
            nw_sb = cpool.tile([1, HID], f32)
            nc.sync.dma_start(nw_sb[:], nw_dr[:])
            nb_sb = cpool.tile([1, HID], f32)
            nc.sync.dma_start(nb_sb[:], nb_dr[:])
            ones32 = cpool.tile([1, P], f32)
            nc.vector.memset(ones32[:], 1.0)
            ps_w = psW.tile([P, HID], f32, space="PSUM", tag="pw")
            nc.tensor.matmul(out=ps_w[:], lhsT=ones32[:], rhs=nw_sb[:],
                             start=True, stop=True)
            w_rep = cpool.tile([P, HID], f32)
            nc.vector.tensor_copy(w_rep[:], ps_w[:])
            ps_b = psW.tile([P, HID], f32, space="PSUM", tag="pw")
            nc.tensor.matmul(out=ps_b[:], lhsT=ones32[:], rhs=nb_sb[:],
                             start=True, stop=True)
            b_rep = cpool.tile([P, HID], f32)
            nc.vector.tensor_copy(b_rep[:], ps_b[:])

        # ---- xsrc table (lo half first so lo gathers can start) -----------
        for b in range((n_lo + n_hi) // BLK):
            nt = bpool.tile([HID, BLK], f16, tag="nt")
            nc.sync.dma_start(nt[:], nodeT[:, b * BLK:(b + 1) * BLK])
            xt = bpool.tile([P, 8, HID], f16, tag="xt")
            for sub in range(2):
                # rides the 2-bank comb pool (same shape/tag) so the block
                # pipeline isn't strangled by a single PSUM bank
                ps = psC.tile([P, 4, HID], f32, space="PSUM", tag="pc")
                for j in range(4):
                    nc.tensor.matmul(
                        out=ps[:, j, :],
                        lhsT=nt[:, sub * 512 + j * P: sub * 512 + (j + 1) * P],
                        rhs=Ws[:], start=True, stop=True,
                        skip_group_check=True)
                if sub == 0:
                    nc.scalar.activation(xt[:, :4, :], ps[:], AF.Copy)
                else:
                    nc.vector.tensor_copy(xt[:, 4:, :], ps[:])
            r0 = b * BLK
            tab, base = (xsrc_lo, 0) if r0 < n_lo else (xsrc_hi, n_lo)
            # table writes ride the ACT hwdge queue; SP carries the reads
            nc.scalar.dma_start(
                tab[r0 - base:r0 - base + BLK, :]
                .rearrange("(s p) h -> p s h", p=P), xt[:])

        # ---- slabs --------------------------------------------------------
        yall = ypool.tile([WIN, nw, HID], f32, tag="yall")
        mvall = ypool.tile([WIN, nw, 2], f32, tag="mvall")
        off16 = {0: 0, 1: 0}
        qn = [0]

        def nextq():
            qn[0] = (qn[0] + 1) % 4
            return qn[0]

        UNIT = 8
        slab_state = {}

        def stage_prep(s):
            """xdw_ext tiles (rows 0:WIN = x_dst via W_dst, WIN:128 = emb)."""
            sl = sched["slabs"][s]
            xdw_t = []
            for wl, w in enumerate(sl["windows"]):
                px = psW.tile([P, HID], f32, space="PSUM", tag="pw")
                nc.tensor.matmul(out=px[:],
                                 lhsT=noT_all[:, w * P:(w + 1) * P],
                                 rhs=Wd[:], start=True, stop=False,
                                 skip_group_check=True)
                nc.tensor.matmul(out=px[:], lhsT=sel8[:], rhs=emb_sb[:],
                                 start=False, stop=True,
                                 skip_group_check=True)
                xdw = xpool.tile([P, HID], f16, tag="xdw",
                                 name=f"xdw{s}_{wl}")
                nc.scalar.activation(xdw[:], px[:], AF.Copy)
                xdw_t.append(xdw)
            C = len(sl["chunks"])
            win_ps = psA.tile([WIN, g.slab_w, 256], f32, space="PSUM",
                              tag="win", name=f"win{s}")
            nc.scalar.memzero(win_ps[:])
            c0 = sl["chunk0"]
            ohxT = spool.tile([P, C, P], mybir.dt.float8e4, tag="ohT",
                              name=f"ohT{s}")
            nc.sync.dma_start(ohxT[:],
                              ohT_in[:, c0 * P:(c0 + C) * P]
                              .rearrange("p (c e) -> p c e", e=P))
            slab_state[s] = dict(
                xdw_t=xdw_t,
                xs_t=spool.tile([P, C, HID], f16, tag="xs", name=f"xs{s}"),
                ohx=spool.tile([P, C, P], mybir.dt.float8e4, tag="ohx",
                               name=f"ohx{s}"),
                win_ps=win_ps, ohxT=ohxT,
            )

        def stage_gather(s):
            sl = sched["slabs"][s]
            xs_t = slab_state[s]["xs_t"]
            for h, (idx_sb, tab, tabn) in enumerate(
                    ((lo_sb, xsrc_lo, n_lo), (hi_sb, xsrc_hi, n_hi))):
                for (slot_off, n) in sl["calls"][h]:
                    if n == 0:
                        continue
                    nc.gpsimd.dma_gather(
                        out_ap=xs_t[:, slot_off:slot_off + n // P, :],
                        in_ap=tab[0:tabn, :],
                        idxs_ap=idx_sb[:, off16[h]:off16[h] + n // 16],
                        num_idxs=n, num_idxs_reg=n, elem_size=HID,
                        single_packet=(n <= 1024), queue_num=nextq(),
                    )
                    off16[h] += n // 16

        def stage_ohx(s):
            """Edge-major one-hot (scatter lhsT), host-built, via DMA."""
            sl = sched["slabs"][s]
            C = len(sl["chunks"])
            c0 = sl["chunk0"]
            ohx = slab_state[s]["ohx"]
            nc.sync.dma_start(ohx[:],
                              oh_in[:, c0 * P:(c0 + C) * P]
                              .rearrange("p (c e) -> p c e", e=P))

        def stage_units(s):
            sl = sched["slabs"][s]
            chunks = sl["chunks"]
            C = len(chunks)
            st = slab_state[s]
            xs_t, ohx, xdw_t = st["xs_t"], st["ohx"], st["xdw_t"]
            lr = spool.tile([P, C, HID], f16, tag="lr", name=f"lr{s}")
            rhs_t = spool.tile([P, C, 4 + HID], f16, tag="rhs", name=f"rhs{s}")
            logits = gpool.tile([P, C, H], f16, tag="logit", name=f"lg{s}")
            st["rhs_t"] = rhs_t
            st["lr"] = lr

            ohxT = st["ohxT"]

            def unit_pe(u0, un):
                # comb psum + Prelu, per 4-chunk psum group (ohxT host-built)
                for g0 in range(u0, u0 + un, 4):
                    gn = min(4, u0 + un - g0)
                    psc = psC.tile([P, 4, HID], f32, space="PSUM", tag="pc")
                    for k in range(gn):
                        wl = chunks[g0 + k][0]
                        nc.tensor.matmul(out=psc[:, k, :],
                                         lhsT=ohxT[:, g0 + k, :],
                                         rhs=xdw_t[wl][:], start=True,
                                         stop=False, skip_group_check=True)
                        nc.tensor.matmul(out=psc[:, k, :], lhsT=ident[:],
                                         rhs=xs_t[:, g0 + k, :], start=False,
                                         stop=True, skip_group_check=True)
                    nc.scalar.activation(lr[:, g0:g0 + gn, :], psc[:, :gn, :],
                                         AF.Prelu, alpha=0.2)

            def unit_post(u0, un):
                # logits / softmax-numerator / weighted messages
                lr4 = lr[:, u0:u0 + un, :].rearrange(
                    "p c (h d) -> p c h d", h=H)
                wgt4 = rhs_t[:, u0:u0 + un, 4:4 + HID].rearrange(
                    "p c (h d) -> p c h d", h=H)
                nc.vector.tensor_mul(
                    wgt4, lr4,
                    att_rep[:].rearrange("p (h d) -> p h d", h=H).unsqueeze(1)
                    .broadcast_to([P, un, H, HD]))
                # tree-fold before the (slow-mode) reduce: adds run at 2x
                nc.vector.tensor_add(wgt4[:, :, :, 0:16], wgt4[:, :, :, 0:16],
                                     wgt4[:, :, :, 16:32])
                nc.vector.tensor_add(wgt4[:, :, :, 0:8], wgt4[:, :, :, 0:8],
                                     wgt4[:, :, :, 8:16])
                with nc.allow_low_precision(reason="logits |x|<12, f16 ok"):
                    nc.vector.tensor_reduce(
                        logits[:, u0:u0 + un, :], wgt4[:, :, :, 0:8],
                        axis=mybir.AxisListType.X, op=OP.add)
                # exp with broadcast INPUT: ex_rep comes out pre-expanded to
                # 32 cols/head so the weighted mul keeps packed operands
                ex_rep = gpool.tile([P, UNIT, HID], f16, tag="exrep")
                nc.scalar.activation(
                    ex_rep[:, :un, :],
                    logits[:, u0:u0 + un, :].unsqueeze(3)
                    .broadcast_to([P, un, H, HD]),
                    AF.Exp, bias=shift_col[:])
                nc.vector.tensor_copy(
                    rhs_t[:, u0:u0 + un, 0:4],
                    ex_rep[:, :un, :].rearrange(
                        "p c (h d) -> p c h d", h=H)[:, :, :, 0])
                nc.vector.tensor_mul(
                    rhs_t[:, u0:u0 + un, 4:4 + HID],
                    xs_t[:, u0:u0 + un, :], ex_rep[:, :un, :])

            # scatter: the bank was memset in stage_prep; all matmuls
            # accumulate (start=False), stop=True on each window's last
            # chunk. Interleaving accumulate-mode writes in one bank is
            # safe (unlike interleaved start=True groups, which corrupt).
            win_ps = st["win_ps"]
            last_slot = {}
            for (wl, h, slot) in chunks:
                last_slot[wl] = slot

            def unit_scatter(u0, un):
                for slot in range(u0, u0 + un):
                    wl = chunks[slot][0]
                    nc.tensor.matmul(out=win_ps[:, wl, 0:4 + HID],
                                     lhsT=ohx[:, slot, 0:WIN],
                                     rhs=rhs_t[:, slot, :],
                                     start=False,
                                     stop=(last_slot[wl] == slot),
                                     skip_group_check=True)

            units = [(u0, min(UNIT, C - u0)) for u0 in range(0, C, UNIT)]
            pend = []
            for (u0, un) in units:
                unit_pe(u0, un)
                if pend:
                    unit_post(*pend[-1])
                if len(pend) >= 2:
                    unit_scatter(*pend[-2])  # two-unit lag: zero PE stall
                pend.append((u0, un))
            unit_post(*pend[-1])
            for (u0, un) in pend[-2:]:
                unit_scatter(u0, un)

        def stage_flush(s):
            sl = sched["slabs"][s]
            ws = sl["windows"]
            nwin = len(ws)
            st = slab_state[s]
            win_ps = st["win_ps"]
            w0 = ws[0]
            if debug:
                C = len(sl["chunks"])
                nc.sync.dma_start(dbg_xs[s, :, :C, :], st["xs_t"][:])
                nc.sync.dma_start(dbg_ohx[s, :, :, :C],
                                  st["ohx"][:].rearrange("p c e -> p e c"))
                nc.sync.dma_start(dbg_lr[s, :, :C, :], st["lr"][:])
                nc.sync.dma_start(dbg_rhs[s, :, :C, :], st["rhs_t"][:])
                for wl, w in enumerate(ws):
                    nc.sync.dma_start(dbg_xdw[w], st["xdw_t"][wl][:])
                    dbw = fpool.tile([WIN, 4 + HID], f32, tag="dbw")
                    nc.vector.tensor_copy(dbw[:], win_ps[:, wl, :])
                    nc.sync.dma_start(dbg_win[w], dbw[:])
            # attention normalization, batched across the slab's windows
            sums = fpool.tile([WIN, g.slab_w, 4], f32, tag="sums")
            nc.vector.tensor_scalar(sums[:, :nwin, :], win_ps[:, :nwin, 0:4],
                                    1e-12, None, OP.max)
            rec = fpool.tile([WIN, g.slab_w, 4], f32, tag="rec")
            nc.vector.reciprocal(rec[:, :nwin, :], sums[:, :nwin, :])
            aggn = fpool.tile([P, g.slab_w, HID], f16, tag="aggn")
            nc.vector.tensor_mul(
                aggn[:WIN, :nwin, :].rearrange("p w (h d) -> p w h d", h=H),
                win_ps[:, :nwin, 4:4 + HID]
                .rearrange("p w (h d) -> p w h d", h=H),
                rec[:, :nwin, :].unsqueeze(3).broadcast_to([WIN, nwin, H, HD]))
            # all windows' agg transposes into one PSUM tile, one copy out
            psTa = psG.tile([P, 4, P], f16, space="PSUM", tag="pg")
            for wl in range(nwin):
                nc.tensor.transpose(out=psTa[:, wl, :], in_=aggn[:, wl, :],
                                    identity=ident[:])
            aggT = fpool.tile([HID, g.slab_w, P], f16, tag="aggT")
            nc.scalar.activation(aggT[:, :nwin, :], psTa[:, :nwin, :],
                                 AF.Copy)
            for wl, w in enumerate(ws):
                # po reuses the window's own PSUM region (agg already read)
                po = win_ps[:, wl, 4:4 + HID]
                nc.tensor.matmul(out=po, lhsT=aggT[:, wl, 0:WIN], rhs=Wosc[:],
                                 start=True, stop=False, skip_group_check=True)
                nc.tensor.matmul(out=po, lhsT=ones_row[:, 0:WIN],
                                 rhs=beta16[:], start=False, stop=True,
                                 skip_group_check=True)
            y = fpool.tile([WIN, g.slab_w, HID], f32, tag="y")
            nc.vector.tensor_add(y[:, :nwin, :], win_ps[:, :nwin, 4:4 + HID],
                                 node_own_sb[:, w0:w0 + nwin, :])
            for wl, w in enumerate(ws):
                st6 = fpool.tile([WIN, 6], f32, tag="st6")
                nc.vector.bn_stats(st6[:], y[:, wl, :])
                nc.vector.bn_aggr(mvall[:, w, :], st6[:])
                if debug:
                    nc.sync.dma_start(dbg_y[w], y[:, wl, :])
            nc.vector.tensor_sub(
                yall[:, w0:w0 + nwin, :], y[:, :nwin, :],
                mvall[:, w0:w0 + nwin, 0:1].broadcast_to([WIN, nwin, HID]))

        nslab = len(sched["slabs"])
        stage_prep(0)
        stage_gather(0)
        stage_ohx(0)
        for s in range(nslab):
            stage_units(s)
            if s + 1 < nslab:
                stage_prep(s + 1)
                stage_gather(s + 1)
                stage_ohx(s + 1)
            stage_flush(s)

        # ---- tail: LN scale + output (keeps Ln/Exp table swap off the
        # slab loop's ACT stream, and the out DMAs off the SP queue) --------
        lnv = fpool.tile([WIN, nw], f32, tag="lnv")
        nc.scalar.activation(lnv[:], mvall[:, :, 1], AF.Ln,
                             bias=eps_col[:WIN, :])
        rstd = fpool.tile([WIN, nw], f32, tag="rstd")
        nc.scalar.activation(rstd[:], lnv[:], AF.Exp, scale=-0.5)
        QW = 14                       # windows per tail chunk
        for q0 in range(0, nw, QW):
            qn = min(QW, nw - q0)
            nc.vector.tensor_mul(
                yall[:, q0:q0 + qn, :], yall[:, q0:q0 + qn, :],
                rstd[:, q0:q0 + qn].unsqueeze(2).broadcast_to([WIN, qn, HID]))
            if not sched["skip_norm"]:
                nc.vector.tensor_mul(
                    yall[:, q0:q0 + qn, :], yall[:, q0:q0 + qn, :],
                    w_rep[:WIN, :].unsqueeze(1).broadcast_to([WIN, qn, HID]))
                nc.vector.tensor_add(
                    yall[:, q0:q0 + qn, :], yall[:, q0:q0 + qn, :],
                    b_rep[:WIN, :].unsqueeze(1).broadcast_to([WIN, qn, HID]))
            r0 = q0 * WIN
            rows = min(npc - r0, qn * WIN)
            nfull = rows // WIN
            if nfull:
                nc.sync.dma_start(
                    out[r0:r0 + nfull * WIN, :]
                    .rearrange("(w p) h -> p w h", p=WIN),
                    yall[:, q0:q0 + nfull, :])
            tail = rows - nfull * WIN
            if tail:
                nc.sync.dma_start(out[r0 + nfull * WIN:r0 + rows, :],
                                  yall[:tail, q0 + nfull, :])

    nc.compile()
    return nc


# ---------------------------------------------------------------------------
_CACHE = {}


def build_all(inputs):
    N = int(np.asarray(inputs["node_embeddings"]).shape[0])
    g = Geo(N=N, n_cores=8)
    sched, in_maps = host_prep(g, **{k: np.asarray(v) for k, v in inputs.items()})
    key = (N, sched["total_chunks"], tuple(int(x) for x in sched["caps"].ravel()),
           sched["skip_norm"], sched["n_lo"], sched["n_hi"])
    if key not in _CACHE:
        _CACHE[key] = build_program(g, sched)
    return g, sched, in_maps, _CACHE[key]


def kernel(**inputs):
    g, sched, in_maps, nc = build_all(inputs)
    from concourse.bass_utils import run_bass_kernel_spmd
    res = run_bass_kernel_spmd(nc, in_maps, core_ids=list(range(g.n_cores)))
    out = np.concatenate([res.results[c]["out"] for c in range(g.n_cores)],
                         axis=0)
    return out.astype(np.float32)
